# revision 7
# baseline (speedup 1.0000x reference)
"""DetectionLoss Bass kernel for Trainium2, data-parallel over 8 NeuronCores.

Strategy (per core, 8 images as 4 image-pairs):
  - layout B: [128 partitions = 2 images x 64 targets, n(preds) free]
  - overlap_x(n,m) = min(relu(x2_n - x1g_m), wg_m) - relu(x1_n - x1g_m)
    relus computed by ScalarE activation (bias = -x1g per partition) while
    evacuating a PE ones-broadcast of the pred-coordinate rows from PSUM.
  - iou > 0.3  <=>  inter > (3/13)(a1+a2); argmax_m iou == argmax_m of
    r = inter * recip((3/13)(a1+a2)) (recip fused into the ScalarE evac).
  - argmax over targets: r cast to bf16, PE-transposed to pred-partition
    space (PSUM), DVE grouped tensor_reduce(max) over the 64-target free
    groups, one-hot mask = (r_T >= best) (exact in bf16; ties averaged via
    a count column), mask PE-transposed back, gather = PE matmul
    (coords hi/lo + count) = GW2^T @ mask.
  - matched flag is NOT carried from the pairwise phase: finalization
    recomputes iou(pred, gathered box) > 0.3, which equals best_iou > 0.3.
  - focal BCE + CIoU finalization in n-partitioned layout, batched over all
    8 images; per-image scalar accumulators reduced via a ones-matmul.
Host combines the 8x8 per-image (focal_sum, masked_ciou_sum, n_pos) triples.
"""

import numpy as np

import concourse.bass as bass
import concourse.bass_isa as bass_isa
from concourse.bacc import Bacc
import concourse.mybir as mybir
from concourse.tile import TileContext

ALU = mybir.AluOpType
ACT = mybir.ActivationFunctionType
F32 = mybir.dt.float32
BF16 = mybir.dt.bfloat16

# problem constants (hardcoded per harness contract)
B_FULL = 64
N = 8400
M = 64
NCORES = 8
BC = B_FULL // NCORES          # images per core
P = 128
C = 66                          # free cols per partition in n-part layout
NPAD = P * C                    # 8448
NC = 1024                       # n-chunk (two PSUM banks of fp32)
CHUNKS = [(k * NC, min(NC, NPAD - k * NC)) for k in range((NPAD + NC - 1) // NC)]
SC13 = 3.0 / 13.0               # iou>0.3  <=>  inter > (3/13)(a1+a2)
EPS = 1e-7


PAD_ROW = np.array([-100.0, -100.0, 1.0, 1.0, -30.0], np.float32)


def pad_preds(preds):
    """Host-side: pad [b, N, 5] -> [b, NPAD, 5] with far-box/low-logit rows."""
    out = np.empty((preds.shape[0], NPAD, 5), np.float32)
    out[:, :N] = preds
    out[:, N:] = PAD_ROW
    return out


def _pred_load(nc, tc, preds_d, PRED, b, bslot, bc):
    """DMA padded preds[b] -> PRED image-slot (n = p*66 + c mapping)."""
    pv = PRED.rearrange("p (b c f) -> p b c f", b=bc, f=5)[:, bslot]  # [128,66,5]
    src = preds_d[b].rearrange("(p c) f -> p c f", c=C)
    nc.sync.dma_start(out=pv[:, :], in_=src)


def host_consts():
    """Host-built constants: selector matmul weights + per-partition scalars."""
    import ml_dtypes
    # K=20 bf16 selector: rows 0..9 hi streams, 10..19 lo streams; stream s
    # picks rows {2s (img A), 2s+1 (img B)} from both halves.
    sels = np.zeros((20, 5 * P), np.float32)
    for s in range(5):
        for base in (0, 10):
            sels[base + 2 * s, s * P : s * P + 64] = 1.0
            sels[base + 2 * s + 1, s * P + 64 : (s + 1) * P] = 1.0
    sels = sels.astype(ml_dtypes.bfloat16)
    onesneg = np.zeros((P, 2), np.float32)
    onesneg[:, 0] = 1.0
    onesneg[:, 1] = -1.0
    identb = np.eye(P, dtype=np.float32).astype(ml_dtypes.bfloat16)
    return sels, onesneg, identb


def build_nc(bc=BC, trn_type=None):
    """Build the per-core Bass program. bc = images per core (even)."""
    pairs = bc // 2
    nc = Bacc() if trn_type is None else Bacc(trn_type=trn_type)
    preds_d = nc.declare_dram_parameter("preds", [bc, NPAD, 5], F32, isOutput=False)
    tgts_d = nc.declare_dram_parameter("targets", [bc, M, 4], F32, isOutput=False)
    sels_d = nc.declare_dram_parameter("sels", [20, 5 * P], BF16, isOutput=False)
    ones_d = nc.declare_dram_parameter("onesneg", [P, 2], F32, isOutput=False)
    identb_d = nc.declare_dram_parameter("identb", [P, P], BF16, isOutput=False)
    out_d = nc.declare_dram_parameter("out", [1, 3 * bc], F32, isOutput=True)

    with TileContext(nc) as tc:
        with (
            tc.tile_pool(name="const", bufs=1) as cpool,
            tc.tile_pool(name="persist", bufs=1) as ppool,
        ):
            # ---- constants (host-supplied) ----
            SELS = cpool.tile([20, 5 * P], BF16, name="SELS")
            nc.sync.dma_start(out=SELS[:, :], in_=sels_d[:, :])
            ON = cpool.tile([P, 2], F32, name="ON")
            nc.sync.dma_start(out=ON[:, :], in_=ones_d[:, :])
            ONES = ON[:, 0:1]
            NEG1 = ON[:, 1:2]
            IDENTB = cpool.tile([P, P], BF16, name="IDENTB")
            nc.sync.dma_start(out=IDENTB[:, :], in_=identb_d[:, :])

            # ---- persistent (all images) ----
            PRED = ppool.tile([P, bc * C * 5], F32, name="PRED")
            X1 = ppool.tile([P, bc * C], F32, name="X1")
            X2 = ppool.tile([P, bc * C], F32, name="X2")
            Y1 = ppool.tile([P, bc * C], F32, name="Y1")
            Y2 = ppool.tile([P, bc * C], F32, name="Y2")
            A1S = ppool.tile([P, bc * C], F32, name="A1S")   # (3/13)*w*h
            # gathered rows per image: 0:4 hi coords, 4 count, 5:9 lo coords,
            # 9 zero  (n = p*66 + c mapping, written back per pair)
            MTF = ppool.tile([P, bc * 10 * C], F32, name="MTF")
            SC = ppool.tile([P, 3 * bc], F32, name="SC")      # accumulator columns

            with (
                tc.tile_pool(name="stage", bufs=1) as spool,
                tc.tile_pool(name="prep", bufs=2) as qpool,
                tc.tile_pool(name="work", bufs=2) as wpool,
                tc.tile_pool(name="psum", bufs=1, space="PSUM") as pspool,
            ):
                # staging rows, shared across pairs:
                #  STGB rows 0..9: hi(x2A,x2B,x1A,x1B,y2A,y2B,y1A,y1B,a1sA,a1sB)
                #       rows 10..19: bf16 lo residuals of the same
                #  STGF rows 0..19: gathered rows (hi coords+count, lo coords)
                STGB = spool.tile([20, NPAD], BF16, name="STGB", bufs=2)
                STGF = spool.tile([20, NPAD], F32, name="STGF", bufs=2)

                for pr in range(pairs):
                    bA, bB = 2 * pr, 2 * pr + 1
                    # ================= prep (n-part layout) =================
                    for bslot in (bA, bB):
                        _pred_load(nc, tc, preds_d, PRED, bslot, bslot, bc)
                    pv = PRED.rearrange("p (b c f) -> p b c f", b=bc, f=5)

                    # per-image coord streams
                    for bslot in (bA, bB):
                        cx = pv[:, bslot, :, 0]
                        cy = pv[:, bslot, :, 1]
                        w = pv[:, bslot, :, 2]
                        h = pv[:, bslot, :, 3]
                        sl = slice(bslot * C, (bslot + 1) * C)
                        WH = qpool.tile([P, C], F32, name="WH", tag="wh", bufs=4)
                        HH = qpool.tile([P, C], F32, name="HH", tag="hh", bufs=4)
                        nc.vector.tensor_scalar(WH[:, :], w, 0.5, None, ALU.mult)
                        nc.vector.tensor_scalar(HH[:, :], h, 0.5, None, ALU.mult)
                        nc.vector.tensor_tensor(X1[:, sl], cx, WH[:, :], ALU.subtract)
                        nc.vector.tensor_tensor(X2[:, sl], cx, WH[:, :], ALU.add)
                        nc.vector.tensor_tensor(Y1[:, sl], cy, HH[:, :], ALU.subtract)
                        nc.vector.tensor_tensor(Y2[:, sl], cy, HH[:, :], ALU.add)
                        nc.vector.scalar_tensor_tensor(
                            A1S[:, sl], w, SC13, h, ALU.mult, ALU.mult
                        )

                    # split to bf16 hi/lo, collapse into rows (n = p*66 + c)
                    for r, T in enumerate((X2, X1, Y2, Y1, A1S)):
                        for j, bslot in enumerate((bA, bB)):
                            tv = T[:, bslot * C : (bslot + 1) * C]
                            THI = qpool.tile([P, C], BF16, name="THI", tag="thi", bufs=4)
                            TLO = qpool.tile([P, C], BF16, name="TLO", tag="tlo", bufs=4)
                            nc.vector.tensor_copy(THI[:, :], tv)
                            nc.vector.tensor_tensor(TLO[:, :], tv, THI[:, :],
                                                    ALU.subtract)
                            for rr, TT_ in ((2 * r + j, THI), (10 + 2 * r + j, TLO)):
                                dst = STGB[rr : rr + 1, :].rearrange(
                                    "o (p c) -> o p c", c=C
                                )
                                nc.sync.dma_start(out=dst, in_=TT_[:, :])

                    # ---- targets: per-partition scalars (A on 0:64, B on 64:128)
                    TGT = qpool.tile([P, 4], F32, name="TGT", tag="tgt", bufs=3)
                    nc.sync.dma_start(out=TGT[0:64, :], in_=tgts_d[bA])
                    nc.sync.dma_start(out=TGT[64:P, :], in_=tgts_d[bB])
                    TWH = qpool.tile([P, 1], F32, name="TWH", tag="twh")
                    THH = qpool.tile([P, 1], F32, name="THH", tag="thh")
                    TX1 = qpool.tile([P, 1], F32, name="TX1", tag="tx1")
                    TY1 = qpool.tile([P, 1], F32, name="TY1", tag="ty1")
                    TX2 = qpool.tile([P, 1], F32, name="TX2", tag="tx2")
                    TY2 = qpool.tile([P, 1], F32, name="TY2", tag="ty2")
                    NX1 = qpool.tile([P, 1], F32, name="NX1", tag="nx1")
                    NY1 = qpool.tile([P, 1], F32, name="NY1", tag="ny1")
                    A2S = qpool.tile([P, 1], F32, name="A2S", tag="a2s")
                    wg = TGT[:, 2:3]
                    hg = TGT[:, 3:4]
                    nc.vector.tensor_scalar(TWH[:, :], wg, 0.5, None, ALU.mult)
                    nc.vector.tensor_scalar(THH[:, :], hg, 0.5, None, ALU.mult)
                    nc.vector.tensor_tensor(TX1[:, :], TGT[:, 0:1], TWH[:, :], ALU.subtract)
                    nc.vector.tensor_tensor(TX2[:, :], TGT[:, 0:1], TWH[:, :], ALU.add)
                    nc.vector.tensor_tensor(TY1[:, :], TGT[:, 1:2], THH[:, :], ALU.subtract)
                    nc.vector.tensor_tensor(TY2[:, :], TGT[:, 1:2], THH[:, :], ALU.add)
                    nc.vector.tensor_scalar(NX1[:, :], TX1[:, :], -1.0, None, ALU.mult)
                    nc.vector.tensor_scalar(NY1[:, :], TY1[:, :], -1.0, None, ALU.mult)
                    nc.vector.scalar_tensor_tensor(
                        A2S[:, :], wg, SC13, hg, ALU.mult, ALU.mult
                    )
                    # gather weights [P, 20] bf16: per image cols {x1,y1,x2,y2,1}
                    # hi at 5j+q (count at 5j+4), lo residuals at 10+5j+q.
                    GW = qpool.tile([P, 10], F32, name="GW", tag="gw", bufs=3)
                    GWB = qpool.tile([P, 20], BF16, name="GWB", tag="gwb", bufs=3)
                    nc.vector.memset(GW[:, :], 0.0)
                    for q, T in enumerate((TX1, TY1, TX2, TY2)):
                        nc.vector.tensor_copy(GW[0:64, q : q + 1], T[0:64, :])
                        nc.vector.tensor_copy(GW[64:P, 5 + q : 6 + q], T[64:P, :])
                    nc.vector.memset(GW[0:64, 4:5], 1.0)
                    nc.vector.memset(GW[64:P, 9:10], 1.0)
                    nc.vector.tensor_copy(GWB[:, 0:10], GW[:, :])
                    nc.vector.tensor_tensor(GWB[:, 10:20], GW[:, :], GWB[:, 0:10],
                                            ALU.subtract)

                    # ================= pairwise chunk loop =================
                    for n0, nc_ in CHUNKS:
                        nbl = nc_ // P  # 128-col transpose blocks in this chunk
                        # PE ones-broadcast of stream rows into PSUM singles
                        PX2 = pspool.tile([P, NC], F32, name="PX2", tag="st", bufs=2)
                        PX1 = pspool.tile([P, NC], F32, name="PX1", tag="st", bufs=2)
                        PY2 = pspool.tile([P, NC], F32, name="PY2", tag="st", bufs=2)
                        PY1 = pspool.tile([P, NC], F32, name="PY1", tag="st", bufs=2)
                        PA1 = pspool.tile([P, NC], F32, name="PA1", tag="st", bufs=2)
                        for j0 in range(0, nc_, 512):
                            jn = min(512, nc_ - j0)
                            rhs = STGB[0:20, n0 + j0 : n0 + j0 + jn]
                            for s, PT_ in enumerate((PX2, PX1, PY2, PY1, PA1)):
                                nc.tensor.matmul(
                                    PT_[:, j0 : j0 + jn],
                                    SELS[:, s * P : (s + 1) * P],
                                    rhs, start=True, stop=True,
                                )
                        # ScalarE: relu with per-partition bias, PSUM -> SBUF
                        AXB = wpool.tile([P, 2 * NC], F32, name="AXB", tag="axb",
                                         bufs=1)
                        AYB = wpool.tile([P, 2 * NC], F32, name="AYB", tag="ayb",
                                         bufs=1)
                        S3 = wpool.tile([P, NC], F32, name="S3", tag="s3", bufs=1)
                        Q = wpool.tile([P, NC], F32, name="Q", tag="q", bufs=1)
                        nc.scalar.activation(AXB[:, 0:nc_], PX2[:, 0:nc_],
                                             ACT.Relu, bias=NX1[:, :])
                        nc.scalar.activation(AXB[:, NC : NC + nc_], PX1[:, 0:nc_],
                                             ACT.Relu, bias=NX1[:, :])
                        nc.scalar.activation(AYB[:, 0:nc_], PY2[:, 0:nc_],
                                             ACT.Relu, bias=NY1[:, :])
                        nc.scalar.activation(AYB[:, NC : NC + nc_], PY1[:, 0:nc_],
                                             ACT.Relu, bias=NY1[:, :])
                        nc.scalar.activation(S3[:, 0:nc_], PA1[:, 0:nc_],
                                             ACT.Identity, bias=A2S[:, :])
                        nc.vector.reciprocal_approx_fast(Q[:, 0:nc_], S3[:, 0:nc_])
                        # DVE: overlaps; GpSimd: relu + inter product
                        CX = wpool.tile([P, NC], F32, name="CX", tag="cx", bufs=1)
                        CY = wpool.tile([P, NC], F32, name="CY", tag="cy", bufs=1)
                        CYR = wpool.tile([P, NC], F32, name="CYR", tag="cyr", bufs=1)
                        INTER = wpool.tile([P, NC], F32, name="INTER", tag="it", bufs=1)
                        RHB = wpool.tile([P, NC], BF16, name="RHB", tag="rh")
                        nc.vector.scalar_tensor_tensor(
                            CX[:, 0:nc_], AXB[:, 0:nc_], wg, AXB[:, NC : NC + nc_],
                            ALU.min, ALU.subtract,
                        )
                        nc.vector.scalar_tensor_tensor(
                            CY[:, 0:nc_], AYB[:, 0:nc_], hg, AYB[:, NC : NC + nc_],
                            ALU.min, ALU.subtract,
                        )
                        nc.scalar.activation(CYR[:, 0:nc_], CY[:, 0:nc_], ACT.Relu)
                        nc.vector.scalar_tensor_tensor(
                            INTER[:, 0:nc_], CX[:, 0:nc_], 0.0, CYR[:, 0:nc_],
                            ALU.max, ALU.mult,
                        )
                        nc.vector.tensor_tensor(RHB[:, 0:nc_], INTER[:, 0:nc_],
                                                Q[:, 0:nc_], ALU.mult)
                        # PE-transpose r (bf16) into pred-partition space
                        RT = pspool.tile([P, NC], BF16, name="RT", tag="rt", bufs=1)
                        for t in range(nbl):
                            nc.tensor.transpose(
                                RT[:, t * P : (t + 1) * P],
                                RHB[:, t * P : (t + 1) * P],
                                IDENTB[:, :],
                            )
                        # best per (pred, image): grouped max over 64 targets
                        BESTC = wpool.tile([P, 2 * NC // P], F32, name="BESTC",
                                           tag="bst")
                        rt4 = RT[:, 0:nc_].rearrange("p (t i m) -> p t i m",
                                                     i=2, m=64)
                        bc3 = BESTC[:, 0 : 2 * nbl].rearrange("p (t i) -> p t i",
                                                              i=2)
                        nc.vector.tensor_reduce(bc3, rt4, mybir.AxisListType.X,
                                                ALU.max)
                        # one-hot (ties allowed; averaged later via count)
                        MASKT = wpool.tile([P, NC], BF16, name="MASKT", tag="mt")
                        mt4 = MASKT[:, 0:nc_].rearrange("p (t i m) -> p t i m",
                                                        i=2, m=64)
                        bb4 = bc3.unsqueeze(3).broadcast_to([P, nbl, 2, 64])
                        nc.vector.tensor_tensor(mt4, rt4, bb4, ALU.is_ge)
                        # transpose mask back to target-partition space
                        MASKM = pspool.tile([P, NC], BF16, name="MASKM", tag="mm",
                                            bufs=1)
                        for t in range(nbl):
                            nc.tensor.transpose(
                                MASKM[:, t * P : (t + 1) * P],
                                MASKT[:, t * P : (t + 1) * P],
                                IDENTB[:, :],
                            )
                        MASKS = wpool.tile([P, NC], BF16, name="MASKS", tag="ms")
                        nc.scalar.activation(MASKS[:, 0:nc_], MASKM[:, 0:nc_],
                                             ACT.Copy)
                        # PE gather: rows = coords hi + count, coords lo
                        GC = pspool.tile([20, NC], F32, name="GC", tag="gc", bufs=1)
                        for j0 in range(0, nc_, 512):
                            jn = min(512, nc_ - j0)
                            nc.tensor.matmul(GC[:, j0 : j0 + jn], GWB[:, :],
                                             MASKS[:, j0 : j0 + jn],
                                             start=True, stop=True)
                        GCB = wpool.tile([20, NC], F32, name="GCB", tag="gcb",
                                         bufs=1)
                        nc.scalar.activation(GCB[:, 0:nc_], GC[:, 0:nc_], ACT.Copy)
                        nc.sync.dma_start(out=STGF[0:20, n0 : n0 + nc_],
                                          in_=GCB[:, 0:nc_])

                    # ============== return to n-part layout ==============
                    # MTF[p, b, rr, c]: rr 0:4 hi coords, 4 count, 5:9 lo coords
                    mtfv = MTF.rearrange("p (b r c) -> p b r c", b=bc, r=10)
                    for j, bslot in enumerate((bA, bB)):
                        for rr in range(5):
                            src = STGF[5 * j + rr : 5 * j + rr + 1, :].rearrange(
                                "o (p c) -> o p c", c=C)
                            nc.sync.dma_start(out=mtfv[:, bslot, rr], in_=src)
                        for q in range(4):
                            r0 = 10 + 5 * j + q
                            src = STGF[r0 : r0 + 1, :].rearrange(
                                "o (p c) -> o p c", c=C)
                            nc.sync.dma_start(out=mtfv[:, bslot, 5 + q], in_=src)

            with (
                tc.tile_pool(name="fin", bufs=1) as wpool,
                tc.tile_pool(name="fpsum", bufs=1, space="PSUM") as pspool,
            ):
                # ================= batched finalization =================
                pv = PRED.rearrange("p (b c f) -> p b c f", b=bc, f=5)
                L = pv[:, :, :, 4]      # logits [128, bc, 66]
                CXp = pv[:, :, :, 0]
                CYp = pv[:, :, :, 1]
                Wp = pv[:, :, :, 2]
                Hp = pv[:, :, :, 3]
                BCC = bc * C

                def ftile(name, tag=None, bufs=None):
                    return wpool.tile([P, BCC], F32, name=name, tag=tag or name,
                                      bufs=bufs or 1)

                # ---- matched-target box: (hi + lo) / count ----
                mtfv = MTF.rearrange("p (b r c) -> p b r c", b=bc, r=10)
                CNTV = mtfv[:, :, 4]                      # [P, bc, C]
                QCN = ftile("QCN")
                nc.vector.reciprocal_approx_fast(
                    QCN.rearrange("p (b c) -> p b c", b=bc), CNTV)
                MTG = wpool.tile([P, bc * 4 * C], F32, name="MTG")
                mtg4 = MTG.rearrange("p (b q c) -> p b q c", b=bc, q=4)
                nc.vector.tensor_tensor(mtg4, mtfv[:, :, 0:4], mtfv[:, :, 5:9],
                                        ALU.add)
                qcb = QCN.rearrange("p (b c) -> p b c", b=bc).unsqueeze(2)
                nc.vector.tensor_tensor(mtg4, mtg4,
                                        qcb.broadcast_to([P, bc, 4, C]), ALU.mult)
                GX1 = mtg4[:, :, 0]
                GY1 = mtg4[:, :, 1]
                GX2 = mtg4[:, :, 2]
                GY2 = mtg4[:, :, 3]
                bview = lambda t: t.rearrange("p (b c) -> p b c", b=bc)

                # ---- intersection with matched boxes + matched flag ----
                T1 = ftile("T1"); T2 = ftile("T2"); T3 = ftile("T3"); T4 = ftile("T4")
                IW = ftile("IW"); IH = ftile("IH"); IN2 = ftile("IN2"); AG = ftile("AG")
                UN = ftile("UN"); QU = ftile("QU"); IOU = ftile("IOU")
                WGE = ftile("WGE"); HGE = ftile("HGE"); A1R = ftile("A1R")
                MTC = ppool.tile([P, BCC], F32, name="MTC")   # matched 0/1

                nc.vector.tensor_tensor(T1[:, :], X1[:, :], GX1, ALU.max)
                nc.vector.tensor_tensor(T2[:, :], X2[:, :], GX2, ALU.min)
                nc.vector.tensor_tensor(IW[:, :], T2[:, :], T1[:, :], ALU.subtract)
                nc.vector.tensor_tensor(T3[:, :], Y1[:, :], GY1, ALU.max)
                nc.vector.tensor_tensor(T4[:, :], Y2[:, :], GY2, ALU.min)
                nc.vector.tensor_tensor(IH[:, :], T4[:, :], T3[:, :], ALU.subtract)
                nc.vector.tensor_scalar(IH[:, :], IH[:, :], 0.0, None, ALU.max)
                nc.vector.scalar_tensor_tensor(IN2[:, :], IW[:, :], 0.0, IH[:, :],
                                               ALU.max, ALU.mult)
                nc.vector.tensor_tensor(WGE[:, :], GX2, GX1, ALU.subtract)
                nc.vector.tensor_tensor(HGE[:, :], GY2, GY1, ALU.subtract)
                nc.vector.tensor_tensor(AG[:, :], WGE[:, :], HGE[:, :], ALU.mult)
                nc.vector.tensor_scalar(A1R[:, :], A1S[:, :], 13.0 / 3.0, None, ALU.mult)
                # matched  <=>  inter > (3/13)(a1 + ag)
                nc.vector.tensor_tensor(T1[:, :], A1R[:, :], AG[:, :], ALU.add)
                nc.vector.scalar_tensor_tensor(MTC[:, :], T1[:, :], SC13, IN2[:, :],
                                               ALU.mult, ALU.is_lt)

                # ---- focal ----
                AZ = ftile("AZ"); SP = ftile("SP"); U0 = ftile("U0"); ZT = ftile("ZT")
                BCE = ftile("BCE"); PT = ftile("PT"); SQ = ftile("SQ"); FF = ftile("FF")
                nc.scalar.activation(AZ[:, :], L, ACT.Abs)
                # softplus(-|z|) = ln(1 + exp(-|z|))
                nc.scalar.activation(SP[:, :], AZ[:, :], ACT.Exp, scale=-1.0)
                nc.scalar.activation(SP[:, :], SP[:, :], ACT.Ln, bias=1.0)
                nc.vector.scalar_tensor_tensor(U0[:, :], L, 0.0, SP[:, :], ALU.max, ALU.add)
                nc.vector.tensor_tensor(ZT[:, :], L, MTC[:, :], ALU.mult)
                nc.vector.tensor_tensor(BCE[:, :], U0[:, :], ZT[:, :], ALU.subtract)
                nc.scalar.activation(PT[:, :], BCE[:, :], ACT.Exp, scale=-1.0)
                nc.scalar.activation(SQ[:, :], PT[:, :], ACT.Square, bias=NEG1[:, :])
                nc.vector.scalar_tensor_tensor(FF[:, :], SQ[:, :], 0.25, BCE[:, :],
                                               ALU.mult, ALU.mult)
                nc.vector.tensor_reduce(SC[:, 0:bc], bview(FF), mybir.AxisListType.X,
                                        ALU.add)

                # ---- CIoU ----
                DX = ftile("DX"); DY = ftile("DY"); DG = ftile("DG"); QD = ftile("QD")
                DD = ftile("DD"); DIOU = ftile("DIOU")
                QH = ftile("QH"); RG = ftile("RG")
                ATG = ftile("ATG"); ATP = ftile("ATP"); VV = ftile("VV"); DEN = ftile("DEN")
                QA = ftile("QA"); AL = ftile("AL"); AV = ftile("AV"); CIO = ftile("CIO")
                MC = ftile("MC")

                # union = a1 + ag - inter
                nc.vector.tensor_tensor(UN[:, :], A1R[:, :], AG[:, :], ALU.add)
                nc.vector.scalar_tensor_tensor(UN[:, :], UN[:, :], EPS, IN2[:, :],
                                               ALU.add, ALU.subtract)
                nc.vector.reciprocal_approx_fast(QU[:, :], UN[:, :])
                nc.vector.tensor_tensor(IOU[:, :], IN2[:, :], QU[:, :], ALU.mult)
                # enclosing diag
                nc.vector.tensor_tensor(T1[:, :], X1[:, :], GX1, ALU.min)
                nc.vector.tensor_tensor(T2[:, :], X2[:, :], GX2, ALU.max)
                nc.vector.tensor_tensor(DX[:, :], T2[:, :], T1[:, :], ALU.subtract)
                nc.vector.tensor_tensor(T3[:, :], Y1[:, :], GY1, ALU.min)
                nc.vector.tensor_tensor(T4[:, :], Y2[:, :], GY2, ALU.max)
                nc.vector.tensor_tensor(DY[:, :], T4[:, :], T3[:, :], ALU.subtract)
                nc.scalar.activation(T1[:, :], DX[:, :], ACT.Square)
                nc.scalar.activation(T2[:, :], DY[:, :], ACT.Square)
                nc.vector.scalar_tensor_tensor(DG[:, :], T1[:, :], EPS, T2[:, :],
                                               ALU.add, ALU.add)
                nc.vector.reciprocal_approx_fast(QD[:, :], DG[:, :])
                # center distance
                nc.vector.tensor_tensor(T3[:, :], GX1, GX2, ALU.add)
                nc.vector.scalar_tensor_tensor(T3[:, :], T3[:, :], 0.5, CXp,
                                               ALU.mult, ALU.subtract)
                nc.vector.tensor_tensor(T4[:, :], GY1, GY2, ALU.add)
                nc.vector.scalar_tensor_tensor(T4[:, :], T4[:, :], 0.5, CYp,
                                               ALU.mult, ALU.subtract)
                nc.scalar.activation(T3[:, :], T3[:, :], ACT.Square)
                nc.scalar.activation(T4[:, :], T4[:, :], ACT.Square)
                nc.vector.tensor_tensor(DD[:, :], T3[:, :], T4[:, :], ALU.add)
                nc.vector.tensor_tensor(DD[:, :], DD[:, :], QD[:, :], ALU.mult)
                # diou - 1 = dist/diag - iou
                nc.vector.scalar_tensor_tensor(DIOU[:, :], IOU[:, :], -1.0, DD[:, :],
                                               ALU.mult, ALU.add)
                # aspect term.  ScalarE Arctan domain is [-pi/2, pi/2], so use
                # arctan(x) = a + 1[x>1]*(pi/2 - 2a),  a = arctan(min(x, 1/x)).
                def atan_pos(dst, x, ta, tb):
                    nc.vector.tensor_scalar(ta[:, :], x[:, :], 1e-20, None, ALU.max)
                    nc.vector.reciprocal_approx_fast(tb[:, :], ta[:, :])
                    nc.vector.tensor_tensor(tb[:, :], ta[:, :], tb[:, :], ALU.min)
                    nc.scalar.activation(dst[:, :], tb[:, :], ACT.Arctan)
                    nc.vector.tensor_scalar(ta[:, :], ta[:, :], 1.0, None, ALU.is_gt)
                    nc.vector.tensor_scalar(tb[:, :], dst[:, :], -2.0, float(np.pi / 2),
                                            ALU.mult, ALU.add)
                    nc.vector.tensor_tensor(ta[:, :], ta[:, :], tb[:, :], ALU.mult)
                    nc.vector.tensor_tensor(dst[:, :], dst[:, :], ta[:, :], ALU.add)

                nc.vector.tensor_scalar(T1[:, :], HGE[:, :], 1e-12, None, ALU.max)
                nc.vector.reciprocal_approx_fast(QH[:, :], T1[:, :])
                nc.vector.tensor_tensor(RG[:, :], WGE[:, :], QH[:, :], ALU.mult)
                atan_pos(ATG, RG, T1, T2)
                nc.vector.tensor_scalar(T2[:, :], Hp, 1e-12, None, ALU.max)
                nc.vector.reciprocal_approx_fast(QH[:, :], T2[:, :])
                nc.vector.scalar_tensor_tensor(RG[:, :], QH[:, :], 1.0, Wp,
                                               ALU.mult, ALU.mult)
                atan_pos(ATP, RG, T1, T2)
                nc.vector.tensor_tensor(T3[:, :], ATG[:, :], ATP[:, :], ALU.subtract)
                nc.scalar.activation(VV[:, :], T3[:, :], ACT.Square,
                                     scale=2.0 / np.pi)
                # alpha = v / (1 - iou + v + eps)
                nc.vector.tensor_tensor(DEN[:, :], VV[:, :], IOU[:, :], ALU.subtract)
                nc.vector.tensor_scalar(DEN[:, :], DEN[:, :], 1.0 + EPS, None, ALU.add)
                nc.vector.reciprocal_approx_fast(QA[:, :], DEN[:, :])
                nc.vector.tensor_tensor(AL[:, :], VV[:, :], QA[:, :], ALU.mult)
                nc.vector.tensor_tensor(AV[:, :], AL[:, :], VV[:, :], ALU.mult)
                # ciou = 1 + (diou - 1) + alpha*v
                nc.vector.scalar_tensor_tensor(CIO[:, :], DIOU[:, :], 1.0, AV[:, :],
                                               ALU.add, ALU.add)
                nc.vector.tensor_tensor(MC[:, :], CIO[:, :], MTC[:, :], ALU.mult)
                nc.vector.tensor_reduce(SC[:, bc : 2 * bc], bview(MC),
                                        mybir.AxisListType.X, ALU.add)
                nc.vector.tensor_reduce(SC[:, 2 * bc : 3 * bc], bview(MTC),
                                        mybir.AxisListType.X, ALU.add)

                # ---- cross-partition reduce + output ----
                PS = pspool.tile([1, 3 * bc], F32, name="PS", tag="ps")
                nc.tensor.matmul(PS[:, :], ONES[:, :], SC[:, :], start=True, stop=True)
                OUTS = wpool.tile([1, 3 * bc], F32, name="OUTS", tag="outs")
                nc.scalar.activation(OUTS[:, :], PS[:, :], ACT.Copy)
                nc.sync.dma_start(out=out_d[:, :], in_=OUTS[:, :])

    nc.finalize()
    return nc


# ---------------- host side ----------------
_CACHE = {}


def _get_nc():
    if "nc" not in _CACHE:
        _CACHE["nc"] = build_nc()
    return _CACHE["nc"]


def combine(per_img):
    """per_img [B, 3] float64: (focal_sum, masked_ciou_sum, n_pos) -> loss."""
    f = per_img[:, 0] / float(N)
    conf = f.mean()
    npos = per_img[:, 2]
    per_box = per_img[:, 1] / np.maximum(npos, 1.0)
    has = (npos > 0).astype(np.float64)
    nimg = has.sum()
    box = (per_box * has).sum() / max(nimg, 1.0)
    return np.float32(conf + 7.5 * box)


def run(preds, targets, **spmd_kwargs):
    from concourse.bass_utils import run_bass_kernel_spmd

    preds = np.ascontiguousarray(preds, np.float32)
    targets = np.ascontiguousarray(targets, np.float32)
    nc = _get_nc()
    sels, onesneg, identb = host_consts()
    in_maps = [
        {
            "preds": pad_preds(preds[c * BC : (c + 1) * BC]),
            "targets": np.ascontiguousarray(targets[c * BC : (c + 1) * BC]),
            "sels": sels,
            "onesneg": onesneg,
            "identb": identb,
        }
        for c in range(NCORES)
    ]
    res = run_bass_kernel_spmd(nc, in_maps, list(range(NCORES)), **spmd_kwargs)
    rows = []
    for c in range(NCORES):
        o = np.asarray(res.results[c]["out"], np.float64).reshape(3, BC)
        rows.append(o.T)  # [BC, 3]
    per_img = np.concatenate(rows, 0)
    return per_img, res


def kernel(preds, targets):
    per_img, _ = run(preds, targets)
    return combine(per_img)


# revision 9
# speedup vs baseline: 1.1230x; 1.1230x over previous
"""DetectionLoss Bass kernel for Trainium2, data-parallel over 8 NeuronCores.

Strategy (per core, 8 images as 4 image-pairs):
  - layout B: [128 partitions = 2 images x 64 targets, n(preds) free]
  - overlap_x(n,m) = min(relu(x2_n - x1g_m), wg_m) - relu(x1_n - x1g_m)
    relus computed by ScalarE activation (bias = -x1g per partition) while
    evacuating a PE ones-broadcast of the pred-coordinate rows from PSUM.
  - iou > 0.3  <=>  inter > (3/13)(a1+a2); argmax_m iou == argmax_m of
    r = inter * recip((3/13)(a1+a2)) (recip fused into the ScalarE evac).
  - argmax over targets: r cast to bf16, PE-transposed to pred-partition
    space (PSUM), DVE grouped tensor_reduce(max) over the 64-target free
    groups, one-hot mask = (r_T >= best) (exact in bf16; ties averaged via
    a count column), mask PE-transposed back, gather = PE matmul
    (coords hi/lo + count) = GW2^T @ mask.
  - matched flag is NOT carried from the pairwise phase: finalization
    recomputes iou(pred, gathered box) > 0.3, which equals best_iou > 0.3.
  - focal BCE + CIoU finalization in n-partitioned layout, batched over all
    8 images; per-image scalar accumulators reduced via a ones-matmul.
Host combines the 8x8 per-image (focal_sum, masked_ciou_sum, n_pos) triples.
"""

import numpy as np

import concourse.bass as bass
import concourse.bass_isa as bass_isa
from concourse.bacc import Bacc
import concourse.mybir as mybir
from concourse.tile import TileContext

ALU = mybir.AluOpType
ACT = mybir.ActivationFunctionType
F32 = mybir.dt.float32
BF16 = mybir.dt.bfloat16

# problem constants (hardcoded per harness contract)
B_FULL = 64
N = 8400
M = 64
NCORES = 8
BC = B_FULL // NCORES          # images per core
P = 128
C = 66                          # free cols per partition in n-part layout
NPAD = P * C                    # 8448
NC = 1024                       # n-chunk (two PSUM banks of fp32)
CHUNKS = [(k * NC, min(NC, NPAD - k * NC)) for k in range((NPAD + NC - 1) // NC)]
SC13 = 3.0 / 13.0               # iou>0.3  <=>  inter > (3/13)(a1+a2)
EPS = 1e-7


PAD_ROW = np.array([-100.0, -100.0, 1.0, 1.0, -30.0], np.float32)


def pad_preds(preds):
    """Host-side: pad [b, N, 5] -> [b, NPAD, 5] with far-box/low-logit rows."""
    out = np.empty((preds.shape[0], NPAD, 5), np.float32)
    out[:, :N] = preds
    out[:, N:] = PAD_ROW
    return out


def _pred_load(nc, tc, preds_d, PRED, b, bslot, bc):
    """DMA padded preds[b] -> PRED image-slot (n = p*66 + c mapping)."""
    pv = PRED.rearrange("p (b c f) -> p b c f", b=bc, f=5)[:, bslot]  # [128,66,5]
    src = preds_d[b].rearrange("(p c) f -> p c f", c=C)
    nc.sync.dma_start(out=pv[:, :], in_=src)


def host_consts():
    """Host-built constants: selector matmul weights + per-partition scalars."""
    import ml_dtypes
    # K=20 bf16 selector: rows 0..9 hi streams, 10..19 lo streams; stream s
    # picks rows {2s (img A), 2s+1 (img B)} from both halves.
    sels = np.zeros((20, 5 * P), np.float32)
    for s in range(5):
        for base in (0, 10):
            sels[base + 2 * s, s * P : s * P + 64] = 1.0
            sels[base + 2 * s + 1, s * P + 64 : (s + 1) * P] = 1.0
    sels = sels.astype(ml_dtypes.bfloat16)
    onesneg = np.zeros((P, 2), np.float32)
    onesneg[:, 0] = 1.0
    onesneg[:, 1] = -1.0
    identb = np.eye(P, dtype=np.float32).astype(ml_dtypes.bfloat16)
    return sels, onesneg, identb


def build_nc(bc=BC, trn_type=None):
    """Build the per-core Bass program. bc = images per core (even)."""
    pairs = bc // 2
    nc = Bacc() if trn_type is None else Bacc(trn_type=trn_type)
    preds_d = nc.declare_dram_parameter("preds", [bc, NPAD, 5], F32, isOutput=False)
    tgts_d = nc.declare_dram_parameter("targets", [bc, M, 4], F32, isOutput=False)
    sels_d = nc.declare_dram_parameter("sels", [20, 5 * P], BF16, isOutput=False)
    ones_d = nc.declare_dram_parameter("onesneg", [P, 2], F32, isOutput=False)
    identb_d = nc.declare_dram_parameter("identb", [P, P], BF16, isOutput=False)
    out_d = nc.declare_dram_parameter("out", [1, 3 * bc], F32, isOutput=True)

    with TileContext(nc) as tc:
        with (
            tc.tile_pool(name="const", bufs=1) as cpool,
            tc.tile_pool(name="persist", bufs=1) as ppool,
        ):
            # ---- constants (host-supplied) ----
            SELS = cpool.tile([20, 5 * P], BF16, name="SELS")
            nc.sync.dma_start(out=SELS[:, :], in_=sels_d[:, :])
            ON = cpool.tile([P, 2], F32, name="ON")
            nc.sync.dma_start(out=ON[:, :], in_=ones_d[:, :])
            ONES = ON[:, 0:1]
            NEG1 = ON[:, 1:2]
            IDENTB = cpool.tile([P, P], BF16, name="IDENTB")
            nc.sync.dma_start(out=IDENTB[:, :], in_=identb_d[:, :])

            # ---- persistent (all images) ----
            PRED = ppool.tile([P, bc * C * 5], F32, name="PRED")
            X1 = ppool.tile([P, bc * C], F32, name="X1")
            X2 = ppool.tile([P, bc * C], F32, name="X2")
            Y1 = ppool.tile([P, bc * C], F32, name="Y1")
            Y2 = ppool.tile([P, bc * C], F32, name="Y2")
            A1S = ppool.tile([P, bc * C], F32, name="A1S")   # (3/13)*w*h
            # gathered rows per image: 0:4 hi coords, 4 count, 5:9 lo coords,
            # 9 zero  (n = p*66 + c mapping, written back per pair)
            MTF = ppool.tile([P, bc * 10 * C], F32, name="MTF")
            SC = ppool.tile([P, 3 * bc], F32, name="SC")      # accumulator columns

            with (
                tc.tile_pool(name="stage", bufs=1) as spool,
                tc.tile_pool(name="prep", bufs=2) as qpool,
                tc.tile_pool(name="work", bufs=2) as wpool,
                tc.tile_pool(name="psum", bufs=1, space="PSUM") as pspool,
            ):
                # staging rows, shared across pairs:
                #  STGB rows 0..9: hi(x2A,x2B,x1A,x1B,y2A,y2B,y1A,y1B,a1sA,a1sB)
                #       rows 10..19: bf16 lo residuals of the same
                #  STGF rows 0..19: gathered rows (hi coords+count, lo coords)
                STGB = spool.tile([20, NPAD], BF16, name="STGB", bufs=2)
                STGF = spool.tile([20, NPAD], F32, name="STGF", bufs=1)

                for pr in range(pairs):
                    bA, bB = 2 * pr, 2 * pr + 1
                    # ================= prep (n-part layout) =================
                    for bslot in (bA, bB):
                        _pred_load(nc, tc, preds_d, PRED, bslot, bslot, bc)
                    pv = PRED.rearrange("p (b c f) -> p b c f", b=bc, f=5)

                    # per-image coord streams
                    for bslot in (bA, bB):
                        cx = pv[:, bslot, :, 0]
                        cy = pv[:, bslot, :, 1]
                        w = pv[:, bslot, :, 2]
                        h = pv[:, bslot, :, 3]
                        sl = slice(bslot * C, (bslot + 1) * C)
                        WH = qpool.tile([P, C], F32, name="WH", tag="wh", bufs=4)
                        HH = qpool.tile([P, C], F32, name="HH", tag="hh", bufs=4)
                        nc.vector.tensor_scalar(WH[:, :], w, 0.5, None, ALU.mult)
                        nc.vector.tensor_scalar(HH[:, :], h, 0.5, None, ALU.mult)
                        nc.vector.tensor_tensor(X1[:, sl], cx, WH[:, :], ALU.subtract)
                        nc.vector.tensor_tensor(X2[:, sl], cx, WH[:, :], ALU.add)
                        nc.vector.tensor_tensor(Y1[:, sl], cy, HH[:, :], ALU.subtract)
                        nc.vector.tensor_tensor(Y2[:, sl], cy, HH[:, :], ALU.add)
                        nc.vector.scalar_tensor_tensor(
                            A1S[:, sl], w, SC13, h, ALU.mult, ALU.mult
                        )

                    # split to bf16 hi/lo, collapse into rows (n = p*66 + c)
                    for r, T in enumerate((X2, X1, Y2, Y1, A1S)):
                        for j, bslot in enumerate((bA, bB)):
                            tv = T[:, bslot * C : (bslot + 1) * C]
                            THI = qpool.tile([P, C], BF16, name="THI", tag="thi", bufs=4)
                            TLO = qpool.tile([P, C], BF16, name="TLO", tag="tlo", bufs=4)
                            nc.vector.tensor_copy(THI[:, :], tv)
                            nc.vector.tensor_tensor(TLO[:, :], tv, THI[:, :],
                                                    ALU.subtract)
                            for rr, TT_ in ((2 * r + j, THI), (10 + 2 * r + j, TLO)):
                                dst = STGB[rr : rr + 1, :].rearrange(
                                    "o (p c) -> o p c", c=C
                                )
                                nc.sync.dma_start(out=dst, in_=TT_[:, :])

                    # ---- targets: per-partition scalars (A on 0:64, B on 64:128)
                    TGT = qpool.tile([P, 4], F32, name="TGT", tag="tgt", bufs=3)
                    nc.sync.dma_start(out=TGT[0:64, :], in_=tgts_d[bA])
                    nc.sync.dma_start(out=TGT[64:P, :], in_=tgts_d[bB])
                    TWH = qpool.tile([P, 1], F32, name="TWH", tag="twh")
                    THH = qpool.tile([P, 1], F32, name="THH", tag="thh")
                    TX1 = qpool.tile([P, 1], F32, name="TX1", tag="tx1")
                    TY1 = qpool.tile([P, 1], F32, name="TY1", tag="ty1")
                    TX2 = qpool.tile([P, 1], F32, name="TX2", tag="tx2")
                    TY2 = qpool.tile([P, 1], F32, name="TY2", tag="ty2")
                    NX1 = qpool.tile([P, 1], F32, name="NX1", tag="nx1")
                    NY1 = qpool.tile([P, 1], F32, name="NY1", tag="ny1")
                    A2S = qpool.tile([P, 1], F32, name="A2S", tag="a2s")
                    wg = TGT[:, 2:3]
                    hg = TGT[:, 3:4]
                    nc.vector.tensor_scalar(TWH[:, :], wg, 0.5, None, ALU.mult)
                    nc.vector.tensor_scalar(THH[:, :], hg, 0.5, None, ALU.mult)
                    nc.vector.tensor_tensor(TX1[:, :], TGT[:, 0:1], TWH[:, :], ALU.subtract)
                    nc.vector.tensor_tensor(TX2[:, :], TGT[:, 0:1], TWH[:, :], ALU.add)
                    nc.vector.tensor_tensor(TY1[:, :], TGT[:, 1:2], THH[:, :], ALU.subtract)
                    nc.vector.tensor_tensor(TY2[:, :], TGT[:, 1:2], THH[:, :], ALU.add)
                    nc.vector.tensor_scalar(NX1[:, :], TX1[:, :], -1.0, None, ALU.mult)
                    nc.vector.tensor_scalar(NY1[:, :], TY1[:, :], -1.0, None, ALU.mult)
                    nc.vector.scalar_tensor_tensor(
                        A2S[:, :], wg, SC13, hg, ALU.mult, ALU.mult
                    )
                    # gather weights [P, 20] bf16: per image cols {x1,y1,x2,y2,1}
                    # hi at 5j+q (count at 5j+4), lo residuals at 10+5j+q.
                    GW = qpool.tile([P, 10], F32, name="GW", tag="gw", bufs=3)
                    GWB = qpool.tile([P, 20], BF16, name="GWB", tag="gwb", bufs=3)
                    nc.vector.memset(GW[:, :], 0.0)
                    for q, T in enumerate((TX1, TY1, TX2, TY2)):
                        nc.vector.tensor_copy(GW[0:64, q : q + 1], T[0:64, :])
                        nc.vector.tensor_copy(GW[64:P, 5 + q : 6 + q], T[64:P, :])
                    nc.vector.memset(GW[0:64, 4:5], 1.0)
                    nc.vector.memset(GW[64:P, 9:10], 1.0)
                    nc.vector.tensor_copy(GWB[:, 0:10], GW[:, :])
                    nc.vector.tensor_tensor(GWB[:, 10:20], GW[:, :], GWB[:, 0:10],
                                            ALU.subtract)

                    # ================= pairwise chunk loop =================
                    for n0, nc_ in CHUNKS:
                        nbl = nc_ // P  # 128-col transpose blocks in this chunk
                        # PE ones-broadcast of stream rows into PSUM singles
                        PX2 = pspool.tile([P, NC], F32, name="PX2", tag="st", bufs=2)
                        PX1 = pspool.tile([P, NC], F32, name="PX1", tag="st", bufs=2)
                        PY2 = pspool.tile([P, NC], F32, name="PY2", tag="st", bufs=2)
                        PY1 = pspool.tile([P, NC], F32, name="PY1", tag="st", bufs=2)
                        PA1 = pspool.tile([P, NC], F32, name="PA1", tag="st", bufs=2)
                        for j0 in range(0, nc_, 512):
                            jn = min(512, nc_ - j0)
                            rhs = STGB[0:20, n0 + j0 : n0 + j0 + jn]
                            for s, PT_ in enumerate((PX2, PX1, PY2, PY1, PA1)):
                                nc.tensor.matmul(
                                    PT_[:, j0 : j0 + jn],
                                    SELS[:, s * P : (s + 1) * P],
                                    rhs, start=True, stop=True,
                                )
                        # ScalarE: relu with per-partition bias, PSUM -> SBUF
                        AXB = wpool.tile([P, 2 * NC], F32, name="AXB", tag="axb",
                                         bufs=2)
                        AYB = wpool.tile([P, 2 * NC], F32, name="AYB", tag="ayb",
                                         bufs=2)
                        S3 = wpool.tile([P, NC], F32, name="S3", tag="s3", bufs=2)
                        Q = wpool.tile([P, NC], F32, name="Q", tag="q", bufs=2)
                        nc.scalar.activation(AXB[:, 0:nc_], PX2[:, 0:nc_],
                                             ACT.Relu, bias=NX1[:, :])
                        nc.scalar.activation(AXB[:, NC : NC + nc_], PX1[:, 0:nc_],
                                             ACT.Relu, bias=NX1[:, :])
                        nc.scalar.activation(AYB[:, 0:nc_], PY2[:, 0:nc_],
                                             ACT.Relu, bias=NY1[:, :])
                        nc.scalar.activation(AYB[:, NC : NC + nc_], PY1[:, 0:nc_],
                                             ACT.Relu, bias=NY1[:, :])
                        nc.scalar.activation(S3[:, 0:nc_], PA1[:, 0:nc_],
                                             ACT.Identity, bias=A2S[:, :])
                        nc.vector.reciprocal_approx_fast(Q[:, 0:nc_], S3[:, 0:nc_])
                        # DVE: overlaps; GpSimd: relu + inter product
                        CX = wpool.tile([P, NC], F32, name="CX", tag="cx", bufs=2)
                        CY = wpool.tile([P, NC], F32, name="CY", tag="cy", bufs=2)
                        CYR = wpool.tile([P, NC], F32, name="CYR", tag="cyr", bufs=1)
                        INTER = wpool.tile([P, NC], F32, name="INTER", tag="it", bufs=2)
                        RHB = wpool.tile([P, NC], BF16, name="RHB", tag="rh")
                        nc.vector.scalar_tensor_tensor(
                            CX[:, 0:nc_], AXB[:, 0:nc_], wg, AXB[:, NC : NC + nc_],
                            ALU.min, ALU.subtract,
                        )
                        nc.vector.scalar_tensor_tensor(
                            CY[:, 0:nc_], AYB[:, 0:nc_], hg, AYB[:, NC : NC + nc_],
                            ALU.min, ALU.subtract,
                        )
                        nc.scalar.activation(CYR[:, 0:nc_], CY[:, 0:nc_], ACT.Relu)
                        nc.vector.scalar_tensor_tensor(
                            INTER[:, 0:nc_], CX[:, 0:nc_], 0.0, CYR[:, 0:nc_],
                            ALU.max, ALU.mult,
                        )
                        nc.vector.tensor_tensor(RHB[:, 0:nc_], INTER[:, 0:nc_],
                                                Q[:, 0:nc_], ALU.mult)
                        # PE-transpose r (bf16) into pred-partition space
                        RT = pspool.tile([P, NC], BF16, name="RT", tag="rt", bufs=1)
                        for t in range(nbl):
                            nc.tensor.transpose(
                                RT[:, t * P : (t + 1) * P],
                                RHB[:, t * P : (t + 1) * P],
                                IDENTB[:, :],
                            )
                        # best per (pred, image): grouped max over 64 targets
                        BESTC = wpool.tile([P, 2 * NC // P], F32, name="BESTC",
                                           tag="bst")
                        rt4 = RT[:, 0:nc_].rearrange("p (t i m) -> p t i m",
                                                     i=2, m=64)
                        bc3 = BESTC[:, 0 : 2 * nbl].rearrange("p (t i) -> p t i",
                                                              i=2)
                        nc.vector.tensor_reduce(bc3, rt4, mybir.AxisListType.X,
                                                ALU.max)
                        # one-hot (ties allowed; averaged later via count)
                        MASKT = wpool.tile([P, NC], BF16, name="MASKT", tag="mt")
                        mt4 = MASKT[:, 0:nc_].rearrange("p (t i m) -> p t i m",
                                                        i=2, m=64)
                        bb4 = bc3.unsqueeze(3).broadcast_to([P, nbl, 2, 64])
                        nc.vector.tensor_tensor(mt4, rt4, bb4, ALU.is_ge)
                        # transpose mask back to target-partition space
                        MASKM = pspool.tile([P, NC], BF16, name="MASKM", tag="mm",
                                            bufs=1)
                        for t in range(nbl):
                            nc.tensor.transpose(
                                MASKM[:, t * P : (t + 1) * P],
                                MASKT[:, t * P : (t + 1) * P],
                                IDENTB[:, :],
                            )
                        MASKS = wpool.tile([P, NC], BF16, name="MASKS", tag="ms")
                        nc.scalar.activation(MASKS[:, 0:nc_], MASKM[:, 0:nc_],
                                             ACT.Copy)
                        # PE gather: rows = coords hi + count, coords lo
                        GC = pspool.tile([20, NC], F32, name="GC", tag="gc", bufs=1)
                        for j0 in range(0, nc_, 512):
                            jn = min(512, nc_ - j0)
                            nc.tensor.matmul(GC[:, j0 : j0 + jn], GWB[:, :],
                                             MASKS[:, j0 : j0 + jn],
                                             start=True, stop=True)
                        GCB = wpool.tile([20, NC], F32, name="GCB", tag="gcb",
                                         bufs=1)
                        nc.scalar.activation(GCB[:, 0:nc_], GC[:, 0:nc_], ACT.Copy)
                        nc.sync.dma_start(out=STGF[0:20, n0 : n0 + nc_],
                                          in_=GCB[:, 0:nc_])

                    # ============== return to n-part layout ==============
                    # MTF[p, b, rr, c]: rr 0:4 hi coords, 4 count, 5:9 lo coords
                    mtfv = MTF.rearrange("p (b r c) -> p b r c", b=bc, r=10)
                    for j, bslot in enumerate((bA, bB)):
                        for rr in range(5):
                            src = STGF[5 * j + rr : 5 * j + rr + 1, :].rearrange(
                                "o (p c) -> o p c", c=C)
                            nc.sync.dma_start(out=mtfv[:, bslot, rr], in_=src)
                        for q in range(4):
                            r0 = 10 + 5 * j + q
                            src = STGF[r0 : r0 + 1, :].rearrange(
                                "o (p c) -> o p c", c=C)
                            nc.sync.dma_start(out=mtfv[:, bslot, 5 + q], in_=src)

            with (
                tc.tile_pool(name="fin", bufs=1) as wpool,
                tc.tile_pool(name="fpsum", bufs=1, space="PSUM") as pspool,
            ):
                # ================= batched finalization =================
                pv = PRED.rearrange("p (b c f) -> p b c f", b=bc, f=5)
                L = pv[:, :, :, 4]      # logits [128, bc, 66]
                CXp = pv[:, :, :, 0]
                CYp = pv[:, :, :, 1]
                Wp = pv[:, :, :, 2]
                Hp = pv[:, :, :, 3]
                BCC = bc * C

                def ftile(name, tag=None, bufs=None):
                    return wpool.tile([P, BCC], F32, name=name, tag=tag or name,
                                      bufs=bufs or 1)

                # ---- matched-target box: (hi + lo) / count ----
                mtfv = MTF.rearrange("p (b r c) -> p b r c", b=bc, r=10)
                CNTV = mtfv[:, :, 4]                      # [P, bc, C]
                QCN = ftile("QCN")
                nc.vector.reciprocal_approx_fast(
                    QCN.rearrange("p (b c) -> p b c", b=bc), CNTV)
                MTG = wpool.tile([P, bc * 4 * C], F32, name="MTG")
                mtg4 = MTG.rearrange("p (b q c) -> p b q c", b=bc, q=4)
                nc.vector.tensor_tensor(mtg4, mtfv[:, :, 0:4], mtfv[:, :, 5:9],
                                        ALU.add)
                qcb = QCN.rearrange("p (b c) -> p b c", b=bc).unsqueeze(2)
                nc.vector.tensor_tensor(mtg4, mtg4,
                                        qcb.broadcast_to([P, bc, 4, C]), ALU.mult)
                GX1 = mtg4[:, :, 0]
                GY1 = mtg4[:, :, 1]
                GX2 = mtg4[:, :, 2]
                GY2 = mtg4[:, :, 3]
                bview = lambda t: t.rearrange("p (b c) -> p b c", b=bc)

                # ---- intersection with matched boxes + matched flag ----
                T1 = ftile("T1"); T2 = ftile("T2"); T3 = ftile("T3"); T4 = ftile("T4")
                IW = ftile("IW"); IH = ftile("IH"); IN2 = ftile("IN2"); AG = ftile("AG")
                UN = ftile("UN"); QU = ftile("QU"); IOU = ftile("IOU")
                WGE = ftile("WGE"); HGE = ftile("HGE"); A1R = ftile("A1R")
                MTC = ppool.tile([P, BCC], F32, name="MTC")   # matched 0/1

                nc.vector.tensor_tensor(T1[:, :], X1[:, :], GX1, ALU.max)
                nc.vector.tensor_tensor(T2[:, :], X2[:, :], GX2, ALU.min)
                nc.vector.tensor_tensor(IW[:, :], T2[:, :], T1[:, :], ALU.subtract)
                nc.vector.tensor_tensor(T3[:, :], Y1[:, :], GY1, ALU.max)
                nc.vector.tensor_tensor(T4[:, :], Y2[:, :], GY2, ALU.min)
                nc.vector.tensor_tensor(IH[:, :], T4[:, :], T3[:, :], ALU.subtract)
                nc.vector.tensor_scalar(IH[:, :], IH[:, :], 0.0, None, ALU.max)
                nc.vector.scalar_tensor_tensor(IN2[:, :], IW[:, :], 0.0, IH[:, :],
                                               ALU.max, ALU.mult)
                nc.vector.tensor_tensor(WGE[:, :], GX2, GX1, ALU.subtract)
                nc.vector.tensor_tensor(HGE[:, :], GY2, GY1, ALU.subtract)
                nc.vector.tensor_tensor(AG[:, :], WGE[:, :], HGE[:, :], ALU.mult)
                nc.vector.tensor_scalar(A1R[:, :], A1S[:, :], 13.0 / 3.0, None, ALU.mult)
                # matched  <=>  inter > (3/13)(a1 + ag)
                nc.vector.tensor_tensor(T1[:, :], A1R[:, :], AG[:, :], ALU.add)
                nc.vector.scalar_tensor_tensor(MTC[:, :], T1[:, :], SC13, IN2[:, :],
                                               ALU.mult, ALU.is_lt)

                # ---- focal ----
                AZ = ftile("AZ"); SP = ftile("SP"); U0 = ftile("U0"); ZT = ftile("ZT")
                BCE = ftile("BCE"); PT = ftile("PT"); SQ = ftile("SQ"); FF = ftile("FF")
                nc.scalar.activation(AZ[:, :], L, ACT.Abs)
                # softplus(-|z|) = ln(1 + exp(-|z|))
                nc.scalar.activation(SP[:, :], AZ[:, :], ACT.Exp, scale=-1.0)
                nc.scalar.activation(SP[:, :], SP[:, :], ACT.Ln, bias=1.0)
                nc.vector.scalar_tensor_tensor(U0[:, :], L, 0.0, SP[:, :], ALU.max, ALU.add)
                nc.vector.tensor_tensor(ZT[:, :], L, MTC[:, :], ALU.mult)
                nc.vector.tensor_tensor(BCE[:, :], U0[:, :], ZT[:, :], ALU.subtract)
                nc.scalar.activation(PT[:, :], BCE[:, :], ACT.Exp, scale=-1.0)
                nc.scalar.activation(SQ[:, :], PT[:, :], ACT.Square, bias=NEG1[:, :])
                nc.vector.scalar_tensor_tensor(FF[:, :], SQ[:, :], 0.25, BCE[:, :],
                                               ALU.mult, ALU.mult)
                nc.vector.tensor_reduce(SC[:, 0:bc], bview(FF), mybir.AxisListType.X,
                                        ALU.add)

                # ---- CIoU ----
                DX = ftile("DX"); DY = ftile("DY"); DG = ftile("DG"); QD = ftile("QD")
                DD = ftile("DD"); DIOU = ftile("DIOU")
                QH = ftile("QH"); RG = ftile("RG")
                ATG = ftile("ATG"); ATP = ftile("ATP"); VV = ftile("VV"); DEN = ftile("DEN")
                QA = ftile("QA"); AL = ftile("AL"); AV = ftile("AV"); CIO = ftile("CIO")
                MC = ftile("MC")

                # union = a1 + ag - inter
                nc.vector.tensor_tensor(UN[:, :], A1R[:, :], AG[:, :], ALU.add)
                nc.vector.scalar_tensor_tensor(UN[:, :], UN[:, :], EPS, IN2[:, :],
                                               ALU.add, ALU.subtract)
                nc.vector.reciprocal_approx_fast(QU[:, :], UN[:, :])
                nc.vector.tensor_tensor(IOU[:, :], IN2[:, :], QU[:, :], ALU.mult)
                # enclosing diag
                nc.vector.tensor_tensor(T1[:, :], X1[:, :], GX1, ALU.min)
                nc.vector.tensor_tensor(T2[:, :], X2[:, :], GX2, ALU.max)
                nc.vector.tensor_tensor(DX[:, :], T2[:, :], T1[:, :], ALU.subtract)
                nc.vector.tensor_tensor(T3[:, :], Y1[:, :], GY1, ALU.min)
                nc.vector.tensor_tensor(T4[:, :], Y2[:, :], GY2, ALU.max)
                nc.vector.tensor_tensor(DY[:, :], T4[:, :], T3[:, :], ALU.subtract)
                nc.scalar.activation(T1[:, :], DX[:, :], ACT.Square)
                nc.scalar.activation(T2[:, :], DY[:, :], ACT.Square)
                nc.vector.scalar_tensor_tensor(DG[:, :], T1[:, :], EPS, T2[:, :],
                                               ALU.add, ALU.add)
                nc.vector.reciprocal_approx_fast(QD[:, :], DG[:, :])
                # center distance
                nc.vector.tensor_tensor(T3[:, :], GX1, GX2, ALU.add)
                nc.vector.scalar_tensor_tensor(T3[:, :], T3[:, :], 0.5, CXp,
                                               ALU.mult, ALU.subtract)
                nc.vector.tensor_tensor(T4[:, :], GY1, GY2, ALU.add)
                nc.vector.scalar_tensor_tensor(T4[:, :], T4[:, :], 0.5, CYp,
                                               ALU.mult, ALU.subtract)
                nc.scalar.activation(T3[:, :], T3[:, :], ACT.Square)
                nc.scalar.activation(T4[:, :], T4[:, :], ACT.Square)
                nc.vector.tensor_tensor(DD[:, :], T3[:, :], T4[:, :], ALU.add)
                nc.vector.tensor_tensor(DD[:, :], DD[:, :], QD[:, :], ALU.mult)
                # diou - 1 = dist/diag - iou
                nc.vector.scalar_tensor_tensor(DIOU[:, :], IOU[:, :], -1.0, DD[:, :],
                                               ALU.mult, ALU.add)
                # aspect term.  ScalarE Arctan domain is [-pi/2, pi/2], so use
                # arctan(x) = a + 1[x>1]*(pi/2 - 2a),  a = arctan(min(x, 1/x)).
                def atan_pos(dst, x, ta, tb):
                    nc.vector.tensor_scalar(ta[:, :], x[:, :], 1e-20, None, ALU.max)
                    nc.vector.reciprocal_approx_fast(tb[:, :], ta[:, :])
                    nc.vector.tensor_tensor(tb[:, :], ta[:, :], tb[:, :], ALU.min)
                    nc.scalar.activation(dst[:, :], tb[:, :], ACT.Arctan)
                    nc.vector.tensor_scalar(ta[:, :], ta[:, :], 1.0, None, ALU.is_gt)
                    nc.vector.tensor_scalar(tb[:, :], dst[:, :], -2.0, float(np.pi / 2),
                                            ALU.mult, ALU.add)
                    nc.vector.tensor_tensor(ta[:, :], ta[:, :], tb[:, :], ALU.mult)
                    nc.vector.tensor_tensor(dst[:, :], dst[:, :], ta[:, :], ALU.add)

                nc.vector.tensor_scalar(T1[:, :], HGE[:, :], 1e-12, None, ALU.max)
                nc.vector.reciprocal_approx_fast(QH[:, :], T1[:, :])
                nc.vector.tensor_tensor(RG[:, :], WGE[:, :], QH[:, :], ALU.mult)
                atan_pos(ATG, RG, T1, T2)
                nc.vector.tensor_scalar(T2[:, :], Hp, 1e-12, None, ALU.max)
                nc.vector.reciprocal_approx_fast(QH[:, :], T2[:, :])
                nc.vector.scalar_tensor_tensor(RG[:, :], QH[:, :], 1.0, Wp,
                                               ALU.mult, ALU.mult)
                atan_pos(ATP, RG, T1, T2)
                nc.vector.tensor_tensor(T3[:, :], ATG[:, :], ATP[:, :], ALU.subtract)
                nc.scalar.activation(VV[:, :], T3[:, :], ACT.Square,
                                     scale=2.0 / np.pi)
                # alpha = v / (1 - iou + v + eps)
                nc.vector.tensor_tensor(DEN[:, :], VV[:, :], IOU[:, :], ALU.subtract)
                nc.vector.tensor_scalar(DEN[:, :], DEN[:, :], 1.0 + EPS, None, ALU.add)
                nc.vector.reciprocal_approx_fast(QA[:, :], DEN[:, :])
                nc.vector.tensor_tensor(AL[:, :], VV[:, :], QA[:, :], ALU.mult)
                nc.vector.tensor_tensor(AV[:, :], AL[:, :], VV[:, :], ALU.mult)
                # ciou = 1 + (diou - 1) + alpha*v
                nc.vector.scalar_tensor_tensor(CIO[:, :], DIOU[:, :], 1.0, AV[:, :],
                                               ALU.add, ALU.add)
                nc.vector.tensor_tensor(MC[:, :], CIO[:, :], MTC[:, :], ALU.mult)
                nc.vector.tensor_reduce(SC[:, bc : 2 * bc], bview(MC),
                                        mybir.AxisListType.X, ALU.add)
                nc.vector.tensor_reduce(SC[:, 2 * bc : 3 * bc], bview(MTC),
                                        mybir.AxisListType.X, ALU.add)

                # ---- cross-partition reduce + output ----
                PS = pspool.tile([1, 3 * bc], F32, name="PS", tag="ps")
                nc.tensor.matmul(PS[:, :], ONES[:, :], SC[:, :], start=True, stop=True)
                OUTS = wpool.tile([1, 3 * bc], F32, name="OUTS", tag="outs")
                nc.scalar.activation(OUTS[:, :], PS[:, :], ACT.Copy)
                nc.sync.dma_start(out=out_d[:, :], in_=OUTS[:, :])

    nc.finalize()
    return nc


# ---------------- host side ----------------
_CACHE = {}


def _get_nc():
    if "nc" not in _CACHE:
        _CACHE["nc"] = build_nc()
    return _CACHE["nc"]


def combine(per_img):
    """per_img [B, 3] float64: (focal_sum, masked_ciou_sum, n_pos) -> loss."""
    f = per_img[:, 0] / float(N)
    conf = f.mean()
    npos = per_img[:, 2]
    per_box = per_img[:, 1] / np.maximum(npos, 1.0)
    has = (npos > 0).astype(np.float64)
    nimg = has.sum()
    box = (per_box * has).sum() / max(nimg, 1.0)
    return np.float32(conf + 7.5 * box)


def run(preds, targets, **spmd_kwargs):
    from concourse.bass_utils import run_bass_kernel_spmd

    preds = np.ascontiguousarray(preds, np.float32)
    targets = np.ascontiguousarray(targets, np.float32)
    nc = _get_nc()
    sels, onesneg, identb = host_consts()
    in_maps = [
        {
            "preds": pad_preds(preds[c * BC : (c + 1) * BC]),
            "targets": np.ascontiguousarray(targets[c * BC : (c + 1) * BC]),
            "sels": sels,
            "onesneg": onesneg,
            "identb": identb,
        }
        for c in range(NCORES)
    ]
    res = run_bass_kernel_spmd(nc, in_maps, list(range(NCORES)), **spmd_kwargs)
    rows = []
    for c in range(NCORES):
        o = np.asarray(res.results[c]["out"], np.float64).reshape(3, BC)
        rows.append(o.T)  # [BC, 3]
    per_img = np.concatenate(rows, 0)
    return per_img, res


def kernel(preds, targets):
    per_img, _ = run(preds, targets)
    return combine(per_img)


# revision 10
# speedup vs baseline: 1.2266x; 1.0923x over previous
"""DetectionLoss Bass kernel for Trainium2, data-parallel over 8 NeuronCores.

Strategy (per core, 8 images as 4 image-pairs):
  - layout B: [128 partitions = 2 images x 64 targets, n(preds) free]
  - overlap_x(n,m) = min(relu(x2_n - x1g_m), wg_m) - relu(x1_n - x1g_m)
    relus computed by ScalarE activation (bias = -x1g per partition) while
    evacuating a PE ones-broadcast of the pred-coordinate rows from PSUM.
  - iou > 0.3  <=>  inter > (3/13)(a1+a2); argmax_m iou == argmax_m of
    r = inter * recip((3/13)(a1+a2)) (recip fused into the ScalarE evac).
  - argmax over targets: r cast to bf16, PE-transposed to pred-partition
    space (PSUM), DVE grouped tensor_reduce(max) over the 64-target free
    groups, one-hot mask = (r_T >= best) (exact in bf16; ties averaged via
    a count column), mask PE-transposed back, gather = PE matmul
    (coords hi/lo + count) = GW2^T @ mask.
  - matched flag is NOT carried from the pairwise phase: finalization
    recomputes iou(pred, gathered box) > 0.3, which equals best_iou > 0.3.
  - focal BCE + CIoU finalization in n-partitioned layout, batched over all
    8 images; per-image scalar accumulators reduced via a ones-matmul.
Host combines the 8x8 per-image (focal_sum, masked_ciou_sum, n_pos) triples.
"""

import numpy as np

import concourse.bass as bass
import concourse.bass_isa as bass_isa
from concourse.bacc import Bacc
import concourse.mybir as mybir
from concourse.tile import TileContext

ALU = mybir.AluOpType
ACT = mybir.ActivationFunctionType
F32 = mybir.dt.float32
BF16 = mybir.dt.bfloat16

# problem constants (hardcoded per harness contract)
B_FULL = 64
N = 8400
M = 64
NCORES = 8
BC = B_FULL // NCORES          # images per core
P = 128
C = 66                          # free cols per partition in n-part layout
NPAD = P * C                    # 8448
NC = 1024                       # n-chunk (two PSUM banks of fp32)
CHUNKS = [(k * NC, min(NC, NPAD - k * NC)) for k in range((NPAD + NC - 1) // NC)]
SC13 = 3.0 / 13.0               # iou>0.3  <=>  inter > (3/13)(a1+a2)
EPS = 1e-7


PAD_ROW = np.array([-100.0, -100.0, 1.0, 1.0, -30.0], np.float32)


def pad_preds(preds):
    """Host-side: pad [b, N, 5] -> [b, NPAD, 5] with far-box/low-logit rows."""
    out = np.empty((preds.shape[0], NPAD, 5), np.float32)
    out[:, :N] = preds
    out[:, N:] = PAD_ROW
    return out


def _pred_load(nc, tc, preds_d, PRED, b, bslot, bc):
    """DMA padded preds[b] -> PRED image-slot (n = p*66 + c mapping)."""
    pv = PRED.rearrange("p (b c f) -> p b c f", b=bc, f=5)[:, bslot]  # [128,66,5]
    src = preds_d[b].rearrange("(p c) f -> p c f", c=C)
    nc.gpsimd.dma_start(out=pv[:, :], in_=src)


def host_consts():
    """Host-built constants: selector matmul weights + per-partition scalars."""
    import ml_dtypes
    # K=20 bf16 selector: rows 0..9 hi streams, 10..19 lo streams; stream s
    # picks rows {2s (img A), 2s+1 (img B)} from both halves.
    sels = np.zeros((20, 5 * P), np.float32)
    for s in range(5):
        for base in (0, 10):
            sels[base + 2 * s, s * P : s * P + 64] = 1.0
            sels[base + 2 * s + 1, s * P + 64 : (s + 1) * P] = 1.0
    sels = sels.astype(ml_dtypes.bfloat16)
    onesneg = np.zeros((P, 2), np.float32)
    onesneg[:, 0] = 1.0
    onesneg[:, 1] = -1.0
    identb = np.eye(P, dtype=np.float32).astype(ml_dtypes.bfloat16)
    return sels, onesneg, identb


def build_nc(bc=BC, trn_type=None):
    """Build the per-core Bass program. bc = images per core (even)."""
    pairs = bc // 2
    nc = Bacc() if trn_type is None else Bacc(trn_type=trn_type)
    preds_d = nc.declare_dram_parameter("preds", [bc, NPAD, 5], F32, isOutput=False)
    tgts_d = nc.declare_dram_parameter("targets", [bc, M, 4], F32, isOutput=False)
    sels_d = nc.declare_dram_parameter("sels", [20, 5 * P], BF16, isOutput=False)
    ones_d = nc.declare_dram_parameter("onesneg", [P, 2], F32, isOutput=False)
    identb_d = nc.declare_dram_parameter("identb", [P, P], BF16, isOutput=False)
    out_d = nc.declare_dram_parameter("out", [1, 3 * bc], F32, isOutput=True)

    with TileContext(nc) as tc:
        with (
            tc.tile_pool(name="const", bufs=1) as cpool,
            tc.tile_pool(name="persist", bufs=1) as ppool,
        ):
            # ---- constants (host-supplied) ----
            SELS = cpool.tile([20, 5 * P], BF16, name="SELS")
            nc.sync.dma_start(out=SELS[:, :], in_=sels_d[:, :])
            ON = cpool.tile([P, 2], F32, name="ON")
            nc.sync.dma_start(out=ON[:, :], in_=ones_d[:, :])
            ONES = ON[:, 0:1]
            NEG1 = ON[:, 1:2]
            IDENTB = cpool.tile([P, P], BF16, name="IDENTB")
            nc.sync.dma_start(out=IDENTB[:, :], in_=identb_d[:, :])

            # ---- persistent (all images) ----
            PRED = ppool.tile([P, bc * C * 5], F32, name="PRED")
            X1 = ppool.tile([P, bc * C], F32, name="X1")
            X2 = ppool.tile([P, bc * C], F32, name="X2")
            Y1 = ppool.tile([P, bc * C], F32, name="Y1")
            Y2 = ppool.tile([P, bc * C], F32, name="Y2")
            A1S = ppool.tile([P, bc * C], F32, name="A1S")   # (3/13)*w*h
            # gathered rows per image: 0:4 hi coords, 4 count, 5:9 lo coords,
            # 9 zero  (n = p*66 + c mapping, written back per pair)
            MTF = ppool.tile([P, bc * 10 * C], BF16, name="MTF")
            SC = ppool.tile([P, 3 * bc], F32, name="SC")      # accumulator columns

            with (
                tc.tile_pool(name="stage", bufs=1) as spool,
                tc.tile_pool(name="prep", bufs=2) as qpool,
                tc.tile_pool(name="work", bufs=2) as wpool,
                tc.tile_pool(name="psum", bufs=1, space="PSUM") as pspool,
            ):
                # staging rows, shared across pairs:
                #  STGB rows 0..9: hi(x2A,x2B,x1A,x1B,y2A,y2B,y1A,y1B,a1sA,a1sB)
                #       rows 10..19: bf16 lo residuals of the same
                #  STGF rows 0..19: gathered rows (hi coords+count, lo coords)
                STGB = spool.tile([20, NPAD], BF16, name="STGB", bufs=2)
                STGF = spool.tile([20, NPAD], BF16, name="STGF", bufs=2)

                for pr in range(pairs):
                    bA, bB = 2 * pr, 2 * pr + 1
                    # ================= prep (n-part layout) =================
                    for bslot in (bA, bB):
                        _pred_load(nc, tc, preds_d, PRED, bslot, bslot, bc)
                    pv = PRED.rearrange("p (b c f) -> p b c f", b=bc, f=5)

                    # per-image coord streams
                    for bslot in (bA, bB):
                        cx = pv[:, bslot, :, 0]
                        cy = pv[:, bslot, :, 1]
                        w = pv[:, bslot, :, 2]
                        h = pv[:, bslot, :, 3]
                        sl = slice(bslot * C, (bslot + 1) * C)
                        WH = qpool.tile([P, C], F32, name="WH", tag="wh", bufs=4)
                        HH = qpool.tile([P, C], F32, name="HH", tag="hh", bufs=4)
                        nc.vector.tensor_scalar(WH[:, :], w, 0.5, None, ALU.mult)
                        nc.vector.tensor_scalar(HH[:, :], h, 0.5, None, ALU.mult)
                        nc.vector.tensor_tensor(X1[:, sl], cx, WH[:, :], ALU.subtract)
                        nc.vector.tensor_tensor(X2[:, sl], cx, WH[:, :], ALU.add)
                        nc.vector.tensor_tensor(Y1[:, sl], cy, HH[:, :], ALU.subtract)
                        nc.vector.tensor_tensor(Y2[:, sl], cy, HH[:, :], ALU.add)
                        nc.vector.scalar_tensor_tensor(
                            A1S[:, sl], w, SC13, h, ALU.mult, ALU.mult
                        )

                    # split to bf16 hi/lo, collapse into rows (n = p*66 + c)
                    for r, T in enumerate((X2, X1, Y2, Y1, A1S)):
                        for j, bslot in enumerate((bA, bB)):
                            tv = T[:, bslot * C : (bslot + 1) * C]
                            THI = qpool.tile([P, C], BF16, name="THI", tag="thi", bufs=4)
                            TLO = qpool.tile([P, C], BF16, name="TLO", tag="tlo", bufs=4)
                            nc.vector.tensor_copy(THI[:, :], tv)
                            nc.vector.tensor_tensor(TLO[:, :], tv, THI[:, :],
                                                    ALU.subtract)
                            for rr, TT_ in ((2 * r + j, THI), (10 + 2 * r + j, TLO)):
                                dst = STGB[rr : rr + 1, :].rearrange(
                                    "o (p c) -> o p c", c=C
                                )
                                nc.sync.dma_start(out=dst, in_=TT_[:, :])

                    # ---- targets: per-partition scalars (A on 0:64, B on 64:128)
                    TGT = qpool.tile([P, 4], F32, name="TGT", tag="tgt", bufs=3)
                    nc.gpsimd.dma_start(out=TGT[0:64, :], in_=tgts_d[bA])
                    nc.gpsimd.dma_start(out=TGT[64:P, :], in_=tgts_d[bB])
                    TWH = qpool.tile([P, 1], F32, name="TWH", tag="twh")
                    THH = qpool.tile([P, 1], F32, name="THH", tag="thh")
                    TX1 = qpool.tile([P, 1], F32, name="TX1", tag="tx1")
                    TY1 = qpool.tile([P, 1], F32, name="TY1", tag="ty1")
                    TX2 = qpool.tile([P, 1], F32, name="TX2", tag="tx2")
                    TY2 = qpool.tile([P, 1], F32, name="TY2", tag="ty2")
                    NX1 = qpool.tile([P, 1], F32, name="NX1", tag="nx1")
                    NY1 = qpool.tile([P, 1], F32, name="NY1", tag="ny1")
                    A2S = qpool.tile([P, 1], F32, name="A2S", tag="a2s")
                    wg = TGT[:, 2:3]
                    hg = TGT[:, 3:4]
                    nc.vector.tensor_scalar(TWH[:, :], wg, 0.5, None, ALU.mult)
                    nc.vector.tensor_scalar(THH[:, :], hg, 0.5, None, ALU.mult)
                    nc.vector.tensor_tensor(TX1[:, :], TGT[:, 0:1], TWH[:, :], ALU.subtract)
                    nc.vector.tensor_tensor(TX2[:, :], TGT[:, 0:1], TWH[:, :], ALU.add)
                    nc.vector.tensor_tensor(TY1[:, :], TGT[:, 1:2], THH[:, :], ALU.subtract)
                    nc.vector.tensor_tensor(TY2[:, :], TGT[:, 1:2], THH[:, :], ALU.add)
                    nc.vector.tensor_scalar(NX1[:, :], TX1[:, :], -1.0, None, ALU.mult)
                    nc.vector.tensor_scalar(NY1[:, :], TY1[:, :], -1.0, None, ALU.mult)
                    nc.vector.scalar_tensor_tensor(
                        A2S[:, :], wg, SC13, hg, ALU.mult, ALU.mult
                    )
                    # gather weights [P, 20] bf16: per image cols {x1,y1,x2,y2,1}
                    # hi at 5j+q (count at 5j+4), lo residuals at 10+5j+q.
                    GW = qpool.tile([P, 10], F32, name="GW", tag="gw", bufs=3)
                    GWB = qpool.tile([P, 20], BF16, name="GWB", tag="gwb", bufs=3)
                    nc.vector.memset(GW[:, :], 0.0)
                    for q, T in enumerate((TX1, TY1, TX2, TY2)):
                        nc.vector.tensor_copy(GW[0:64, q : q + 1], T[0:64, :])
                        nc.vector.tensor_copy(GW[64:P, 5 + q : 6 + q], T[64:P, :])
                    nc.vector.memset(GW[0:64, 4:5], 1.0)
                    nc.vector.memset(GW[64:P, 9:10], 1.0)
                    nc.vector.tensor_copy(GWB[:, 0:10], GW[:, :])
                    nc.vector.tensor_tensor(GWB[:, 10:20], GW[:, :], GWB[:, 0:10],
                                            ALU.subtract)

                    # ================= pairwise chunk loop =================
                    for n0, nc_ in CHUNKS:
                        nbl = nc_ // P  # 128-col transpose blocks in this chunk
                        # PE ones-broadcast of stream rows into PSUM singles
                        PX2 = pspool.tile([P, NC], F32, name="PX2", tag="st", bufs=2)
                        PX1 = pspool.tile([P, NC], F32, name="PX1", tag="st", bufs=2)
                        PY2 = pspool.tile([P, NC], F32, name="PY2", tag="st", bufs=2)
                        PY1 = pspool.tile([P, NC], F32, name="PY1", tag="st", bufs=2)
                        PA1 = pspool.tile([P, NC], F32, name="PA1", tag="st", bufs=2)
                        for j0 in range(0, nc_, 512):
                            jn = min(512, nc_ - j0)
                            rhs = STGB[0:20, n0 + j0 : n0 + j0 + jn]
                            for s, PT_ in enumerate((PX2, PX1, PY2, PY1, PA1)):
                                nc.tensor.matmul(
                                    PT_[:, j0 : j0 + jn],
                                    SELS[:, s * P : (s + 1) * P],
                                    rhs, start=True, stop=True,
                                )
                        # ScalarE: relu with per-partition bias, PSUM -> SBUF
                        AXB = wpool.tile([P, 2 * NC], F32, name="AXB", tag="axb",
                                         bufs=2)
                        AYB = wpool.tile([P, 2 * NC], F32, name="AYB", tag="ayb",
                                         bufs=2)
                        S3 = wpool.tile([P, NC], F32, name="S3", tag="s3", bufs=2)
                        Q = wpool.tile([P, NC], F32, name="Q", tag="q", bufs=2)
                        nc.scalar.activation(AXB[:, 0:nc_], PX2[:, 0:nc_],
                                             ACT.Relu, bias=NX1[:, :])
                        nc.scalar.activation(AXB[:, NC : NC + nc_], PX1[:, 0:nc_],
                                             ACT.Relu, bias=NX1[:, :])
                        nc.scalar.activation(AYB[:, 0:nc_], PY2[:, 0:nc_],
                                             ACT.Relu, bias=NY1[:, :])
                        nc.scalar.activation(AYB[:, NC : NC + nc_], PY1[:, 0:nc_],
                                             ACT.Relu, bias=NY1[:, :])
                        nc.scalar.activation(S3[:, 0:nc_], PA1[:, 0:nc_],
                                             ACT.Identity, bias=A2S[:, :])
                        nc.vector.reciprocal_approx_fast(Q[:, 0:nc_], S3[:, 0:nc_])
                        # DVE: overlaps; GpSimd: relu + inter product
                        CX = wpool.tile([P, NC], F32, name="CX", tag="cx", bufs=2)
                        CY = wpool.tile([P, NC], F32, name="CY", tag="cy", bufs=2)
                        CYR = wpool.tile([P, NC], F32, name="CYR", tag="cyr", bufs=1)
                        INTER = wpool.tile([P, NC], F32, name="INTER", tag="it", bufs=2)
                        RHB = wpool.tile([P, NC], BF16, name="RHB", tag="rh")
                        nc.vector.scalar_tensor_tensor(
                            CX[:, 0:nc_], AXB[:, 0:nc_], wg, AXB[:, NC : NC + nc_],
                            ALU.min, ALU.subtract,
                        )
                        nc.vector.scalar_tensor_tensor(
                            CY[:, 0:nc_], AYB[:, 0:nc_], hg, AYB[:, NC : NC + nc_],
                            ALU.min, ALU.subtract,
                        )
                        nc.scalar.activation(CYR[:, 0:nc_], CY[:, 0:nc_], ACT.Relu)
                        nc.vector.scalar_tensor_tensor(
                            INTER[:, 0:nc_], CX[:, 0:nc_], 0.0, CYR[:, 0:nc_],
                            ALU.max, ALU.mult,
                        )
                        nc.vector.tensor_tensor(RHB[:, 0:nc_], INTER[:, 0:nc_],
                                                Q[:, 0:nc_], ALU.mult)
                        # PE-transpose r (bf16) into pred-partition space
                        RT = pspool.tile([P, NC], BF16, name="RT", tag="rt", bufs=1)
                        for t in range(nbl):
                            nc.tensor.transpose(
                                RT[:, t * P : (t + 1) * P],
                                RHB[:, t * P : (t + 1) * P],
                                IDENTB[:, :],
                            )
                        # best per (pred, image): grouped max over 64 targets
                        BESTC = wpool.tile([P, 2 * NC // P], F32, name="BESTC",
                                           tag="bst")
                        rt4 = RT[:, 0:nc_].rearrange("p (t i m) -> p t i m",
                                                     i=2, m=64)
                        bc3 = BESTC[:, 0 : 2 * nbl].rearrange("p (t i) -> p t i",
                                                              i=2)
                        nc.vector.tensor_reduce(bc3, rt4, mybir.AxisListType.X,
                                                ALU.max)
                        # one-hot (ties allowed; averaged later via count)
                        MASKT = wpool.tile([P, NC], BF16, name="MASKT", tag="mt")
                        mt4 = MASKT[:, 0:nc_].rearrange("p (t i m) -> p t i m",
                                                        i=2, m=64)
                        bb4 = bc3.unsqueeze(3).broadcast_to([P, nbl, 2, 64])
                        nc.vector.tensor_tensor(mt4, rt4, bb4, ALU.is_ge)
                        # transpose mask back to target-partition space
                        MASKM = pspool.tile([P, NC], BF16, name="MASKM", tag="mm",
                                            bufs=1)
                        for t in range(nbl):
                            nc.tensor.transpose(
                                MASKM[:, t * P : (t + 1) * P],
                                MASKT[:, t * P : (t + 1) * P],
                                IDENTB[:, :],
                            )
                        MASKS = wpool.tile([P, NC], BF16, name="MASKS", tag="ms")
                        nc.scalar.activation(MASKS[:, 0:nc_], MASKM[:, 0:nc_],
                                             ACT.Copy)
                        # PE gather: rows = coords hi + count, coords lo
                        GC = pspool.tile([20, NC], F32, name="GC", tag="gc", bufs=1)
                        for j0 in range(0, nc_, 512):
                            jn = min(512, nc_ - j0)
                            nc.tensor.matmul(GC[:, j0 : j0 + jn], GWB[:, :],
                                             MASKS[:, j0 : j0 + jn],
                                             start=True, stop=True)
                        GCB = wpool.tile([20, NC], BF16, name="GCB", tag="gcb",
                                         bufs=2)
                        nc.scalar.activation(GCB[:, 0:nc_], GC[:, 0:nc_], ACT.Copy)
                        nc.sync.dma_start(out=STGF[0:20, n0 : n0 + nc_],
                                          in_=GCB[:, 0:nc_])

                    # ============== return to n-part layout ==============
                    # MTF[p, b, rr, c]: rr 0:4 hi coords, 4 count, 5:9 lo coords
                    mtfv = MTF.rearrange("p (b r c) -> p b r c", b=bc, r=10)
                    for j, bslot in enumerate((bA, bB)):
                        for rr in range(5):
                            src = STGF[5 * j + rr : 5 * j + rr + 1, :].rearrange(
                                "o (p c) -> o p c", c=C)
                            nc.gpsimd.dma_start(out=mtfv[:, bslot, rr], in_=src)
                        for q in range(4):
                            r0 = 10 + 5 * j + q
                            src = STGF[r0 : r0 + 1, :].rearrange(
                                "o (p c) -> o p c", c=C)
                            nc.gpsimd.dma_start(out=mtfv[:, bslot, 5 + q], in_=src)

            with (
                tc.tile_pool(name="fin", bufs=1) as wpool,
                tc.tile_pool(name="fpsum", bufs=1, space="PSUM") as pspool,
            ):
                # ================= batched finalization =================
                pv = PRED.rearrange("p (b c f) -> p b c f", b=bc, f=5)
                L = pv[:, :, :, 4]      # logits [128, bc, 66]
                CXp = pv[:, :, :, 0]
                CYp = pv[:, :, :, 1]
                Wp = pv[:, :, :, 2]
                Hp = pv[:, :, :, 3]
                BCC = bc * C

                def ftile(name, tag=None, bufs=None):
                    return wpool.tile([P, BCC], F32, name=name, tag=tag or name,
                                      bufs=bufs or 1)

                # ---- matched-target box: (hi + lo) / count ----
                mtfv = MTF.rearrange("p (b r c) -> p b r c", b=bc, r=10)
                CNTV = mtfv[:, :, 4]                      # [P, bc, C]
                CNTF = ftile("CNTF")
                nc.scalar.activation(CNTF.rearrange("p (b c) -> p b c", b=bc),
                                     CNTV, ACT.Copy)
                QCN = ftile("QCN")
                nc.vector.reciprocal_approx_fast(QCN[:, :], CNTF[:, :])
                MTG = wpool.tile([P, bc * 4 * C], F32, name="MTG")
                mtg4 = MTG.rearrange("p (b q c) -> p b q c", b=bc, q=4)
                nc.vector.tensor_tensor(mtg4, mtfv[:, :, 0:4], mtfv[:, :, 5:9],
                                        ALU.add)
                qcb = QCN.rearrange("p (b c) -> p b c", b=bc).unsqueeze(2)
                nc.vector.tensor_tensor(mtg4, mtg4,
                                        qcb.broadcast_to([P, bc, 4, C]), ALU.mult)
                GX1 = mtg4[:, :, 0]
                GY1 = mtg4[:, :, 1]
                GX2 = mtg4[:, :, 2]
                GY2 = mtg4[:, :, 3]
                bview = lambda t: t.rearrange("p (b c) -> p b c", b=bc)

                # ---- intersection with matched boxes + matched flag ----
                T1 = ftile("T1"); T2 = ftile("T2"); T3 = ftile("T3"); T4 = ftile("T4")
                IW = ftile("IW"); IH = ftile("IH"); IN2 = ftile("IN2"); AG = ftile("AG")
                UN = ftile("UN"); QU = ftile("QU"); IOU = ftile("IOU")
                WGE = ftile("WGE"); HGE = ftile("HGE"); A1R = ftile("A1R")
                MTC = ppool.tile([P, BCC], F32, name="MTC")   # matched 0/1

                nc.vector.tensor_tensor(T1[:, :], X1[:, :], GX1, ALU.max)
                nc.vector.tensor_tensor(T2[:, :], X2[:, :], GX2, ALU.min)
                nc.vector.tensor_tensor(IW[:, :], T2[:, :], T1[:, :], ALU.subtract)
                nc.vector.tensor_tensor(T3[:, :], Y1[:, :], GY1, ALU.max)
                nc.vector.tensor_tensor(T4[:, :], Y2[:, :], GY2, ALU.min)
                nc.vector.tensor_tensor(IH[:, :], T4[:, :], T3[:, :], ALU.subtract)
                nc.vector.tensor_scalar(IH[:, :], IH[:, :], 0.0, None, ALU.max)
                nc.vector.scalar_tensor_tensor(IN2[:, :], IW[:, :], 0.0, IH[:, :],
                                               ALU.max, ALU.mult)
                nc.vector.tensor_tensor(WGE[:, :], GX2, GX1, ALU.subtract)
                nc.vector.tensor_tensor(HGE[:, :], GY2, GY1, ALU.subtract)
                nc.vector.tensor_tensor(AG[:, :], WGE[:, :], HGE[:, :], ALU.mult)
                nc.vector.tensor_scalar(A1R[:, :], A1S[:, :], 13.0 / 3.0, None, ALU.mult)
                # matched  <=>  inter > (3/13)(a1 + ag)
                nc.vector.tensor_tensor(T1[:, :], A1R[:, :], AG[:, :], ALU.add)
                nc.vector.scalar_tensor_tensor(MTC[:, :], T1[:, :], SC13, IN2[:, :],
                                               ALU.mult, ALU.is_lt)

                # ---- focal ----
                AZ = ftile("AZ"); SP = ftile("SP"); U0 = ftile("U0"); ZT = ftile("ZT")
                BCE = ftile("BCE"); PT = ftile("PT"); SQ = ftile("SQ"); FF = ftile("FF")
                nc.scalar.activation(AZ[:, :], L, ACT.Abs)
                # softplus(-|z|) = ln(1 + exp(-|z|))
                nc.scalar.activation(SP[:, :], AZ[:, :], ACT.Exp, scale=-1.0)
                nc.scalar.activation(SP[:, :], SP[:, :], ACT.Ln, bias=1.0)
                nc.vector.scalar_tensor_tensor(U0[:, :], L, 0.0, SP[:, :], ALU.max, ALU.add)
                nc.vector.tensor_tensor(ZT[:, :], L, MTC[:, :], ALU.mult)
                nc.vector.tensor_tensor(BCE[:, :], U0[:, :], ZT[:, :], ALU.subtract)
                nc.scalar.activation(PT[:, :], BCE[:, :], ACT.Exp, scale=-1.0)
                nc.scalar.activation(SQ[:, :], PT[:, :], ACT.Square, bias=NEG1[:, :])
                nc.vector.scalar_tensor_tensor(FF[:, :], SQ[:, :], 0.25, BCE[:, :],
                                               ALU.mult, ALU.mult)
                nc.vector.tensor_reduce(SC[:, 0:bc], bview(FF), mybir.AxisListType.X,
                                        ALU.add)

                # ---- CIoU ----
                DX = ftile("DX"); DY = ftile("DY"); DG = ftile("DG"); QD = ftile("QD")
                DD = ftile("DD"); DIOU = ftile("DIOU")
                QH = ftile("QH"); RG = ftile("RG")
                ATG = ftile("ATG"); ATP = ftile("ATP"); VV = ftile("VV"); DEN = ftile("DEN")
                QA = ftile("QA"); AL = ftile("AL"); AV = ftile("AV"); CIO = ftile("CIO")
                MC = ftile("MC")

                # union = a1 + ag - inter
                nc.vector.tensor_tensor(UN[:, :], A1R[:, :], AG[:, :], ALU.add)
                nc.vector.scalar_tensor_tensor(UN[:, :], UN[:, :], EPS, IN2[:, :],
                                               ALU.add, ALU.subtract)
                nc.vector.reciprocal_approx_fast(QU[:, :], UN[:, :])
                nc.vector.tensor_tensor(IOU[:, :], IN2[:, :], QU[:, :], ALU.mult)
                # enclosing diag
                nc.vector.tensor_tensor(T1[:, :], X1[:, :], GX1, ALU.min)
                nc.vector.tensor_tensor(T2[:, :], X2[:, :], GX2, ALU.max)
                nc.vector.tensor_tensor(DX[:, :], T2[:, :], T1[:, :], ALU.subtract)
                nc.vector.tensor_tensor(T3[:, :], Y1[:, :], GY1, ALU.min)
                nc.vector.tensor_tensor(T4[:, :], Y2[:, :], GY2, ALU.max)
                nc.vector.tensor_tensor(DY[:, :], T4[:, :], T3[:, :], ALU.subtract)
                nc.scalar.activation(T1[:, :], DX[:, :], ACT.Square)
                nc.scalar.activation(T2[:, :], DY[:, :], ACT.Square)
                nc.vector.scalar_tensor_tensor(DG[:, :], T1[:, :], EPS, T2[:, :],
                                               ALU.add, ALU.add)
                nc.vector.reciprocal_approx_fast(QD[:, :], DG[:, :])
                # center distance
                nc.vector.tensor_tensor(T3[:, :], GX1, GX2, ALU.add)
                nc.vector.scalar_tensor_tensor(T3[:, :], T3[:, :], 0.5, CXp,
                                               ALU.mult, ALU.subtract)
                nc.vector.tensor_tensor(T4[:, :], GY1, GY2, ALU.add)
                nc.vector.scalar_tensor_tensor(T4[:, :], T4[:, :], 0.5, CYp,
                                               ALU.mult, ALU.subtract)
                nc.scalar.activation(T3[:, :], T3[:, :], ACT.Square)
                nc.scalar.activation(T4[:, :], T4[:, :], ACT.Square)
                nc.vector.tensor_tensor(DD[:, :], T3[:, :], T4[:, :], ALU.add)
                nc.vector.tensor_tensor(DD[:, :], DD[:, :], QD[:, :], ALU.mult)
                # diou - 1 = dist/diag - iou
                nc.vector.scalar_tensor_tensor(DIOU[:, :], IOU[:, :], -1.0, DD[:, :],
                                               ALU.mult, ALU.add)
                # aspect term.  ScalarE Arctan domain is [-pi/2, pi/2], so use
                # arctan(x) = a + 1[x>1]*(pi/2 - 2a),  a = arctan(min(x, 1/x)).
                def atan_pos(dst, x, ta, tb):
                    nc.vector.tensor_scalar(ta[:, :], x[:, :], 1e-20, None, ALU.max)
                    nc.vector.reciprocal_approx_fast(tb[:, :], ta[:, :])
                    nc.vector.tensor_tensor(tb[:, :], ta[:, :], tb[:, :], ALU.min)
                    nc.scalar.activation(dst[:, :], tb[:, :], ACT.Arctan)
                    nc.vector.tensor_scalar(ta[:, :], ta[:, :], 1.0, None, ALU.is_gt)
                    nc.vector.tensor_scalar(tb[:, :], dst[:, :], -2.0, float(np.pi / 2),
                                            ALU.mult, ALU.add)
                    nc.vector.tensor_tensor(ta[:, :], ta[:, :], tb[:, :], ALU.mult)
                    nc.vector.tensor_tensor(dst[:, :], dst[:, :], ta[:, :], ALU.add)

                nc.vector.tensor_scalar(T1[:, :], HGE[:, :], 1e-12, None, ALU.max)
                nc.vector.reciprocal_approx_fast(QH[:, :], T1[:, :])
                nc.vector.tensor_tensor(RG[:, :], WGE[:, :], QH[:, :], ALU.mult)
                atan_pos(ATG, RG, T1, T2)
                nc.vector.tensor_scalar(T2[:, :], Hp, 1e-12, None, ALU.max)
                nc.vector.reciprocal_approx_fast(QH[:, :], T2[:, :])
                nc.vector.scalar_tensor_tensor(RG[:, :], QH[:, :], 1.0, Wp,
                                               ALU.mult, ALU.mult)
                atan_pos(ATP, RG, T1, T2)
                nc.vector.tensor_tensor(T3[:, :], ATG[:, :], ATP[:, :], ALU.subtract)
                nc.scalar.activation(VV[:, :], T3[:, :], ACT.Square,
                                     scale=2.0 / np.pi)
                # alpha = v / (1 - iou + v + eps)
                nc.vector.tensor_tensor(DEN[:, :], VV[:, :], IOU[:, :], ALU.subtract)
                nc.vector.tensor_scalar(DEN[:, :], DEN[:, :], 1.0 + EPS, None, ALU.add)
                nc.vector.reciprocal_approx_fast(QA[:, :], DEN[:, :])
                nc.vector.tensor_tensor(AL[:, :], VV[:, :], QA[:, :], ALU.mult)
                nc.vector.tensor_tensor(AV[:, :], AL[:, :], VV[:, :], ALU.mult)
                # ciou = 1 + (diou - 1) + alpha*v
                nc.vector.scalar_tensor_tensor(CIO[:, :], DIOU[:, :], 1.0, AV[:, :],
                                               ALU.add, ALU.add)
                nc.vector.tensor_tensor(MC[:, :], CIO[:, :], MTC[:, :], ALU.mult)
                nc.vector.tensor_reduce(SC[:, bc : 2 * bc], bview(MC),
                                        mybir.AxisListType.X, ALU.add)
                nc.vector.tensor_reduce(SC[:, 2 * bc : 3 * bc], bview(MTC),
                                        mybir.AxisListType.X, ALU.add)

                # ---- cross-partition reduce + output ----
                PS = pspool.tile([1, 3 * bc], F32, name="PS", tag="ps")
                nc.tensor.matmul(PS[:, :], ONES[:, :], SC[:, :], start=True, stop=True)
                OUTS = wpool.tile([1, 3 * bc], F32, name="OUTS", tag="outs")
                nc.scalar.activation(OUTS[:, :], PS[:, :], ACT.Copy)
                nc.sync.dma_start(out=out_d[:, :], in_=OUTS[:, :])

    nc.finalize()
    return nc


# ---------------- host side ----------------
_CACHE = {}


def _get_nc():
    if "nc" not in _CACHE:
        _CACHE["nc"] = build_nc()
    return _CACHE["nc"]


def combine(per_img):
    """per_img [B, 3] float64: (focal_sum, masked_ciou_sum, n_pos) -> loss."""
    f = per_img[:, 0] / float(N)
    conf = f.mean()
    npos = per_img[:, 2]
    per_box = per_img[:, 1] / np.maximum(npos, 1.0)
    has = (npos > 0).astype(np.float64)
    nimg = has.sum()
    box = (per_box * has).sum() / max(nimg, 1.0)
    return np.float32(conf + 7.5 * box)


def run(preds, targets, **spmd_kwargs):
    from concourse.bass_utils import run_bass_kernel_spmd

    preds = np.ascontiguousarray(preds, np.float32)
    targets = np.ascontiguousarray(targets, np.float32)
    nc = _get_nc()
    sels, onesneg, identb = host_consts()
    in_maps = [
        {
            "preds": pad_preds(preds[c * BC : (c + 1) * BC]),
            "targets": np.ascontiguousarray(targets[c * BC : (c + 1) * BC]),
            "sels": sels,
            "onesneg": onesneg,
            "identb": identb,
        }
        for c in range(NCORES)
    ]
    res = run_bass_kernel_spmd(nc, in_maps, list(range(NCORES)), **spmd_kwargs)
    rows = []
    for c in range(NCORES):
        o = np.asarray(res.results[c]["out"], np.float64).reshape(3, BC)
        rows.append(o.T)  # [BC, 3]
    per_img = np.concatenate(rows, 0)
    return per_img, res


def kernel(preds, targets):
    per_img, _ = run(preds, targets)
    return combine(per_img)


# revision 12
# speedup vs baseline: 1.3632x; 1.1113x over previous
"""DetectionLoss Bass kernel for Trainium2, data-parallel over 8 NeuronCores.

Strategy (per core, 8 images as 4 image-pairs):
  - layout B: [128 partitions = 2 images x 64 targets, n(preds) free]
  - overlap_x(n,m) = min(relu(x2_n - x1g_m), wg_m) - relu(x1_n - x1g_m)
    relus computed by ScalarE activation (bias = -x1g per partition) while
    evacuating a PE ones-broadcast of the pred-coordinate rows from PSUM.
  - iou > 0.3  <=>  inter > (3/13)(a1+a2); argmax_m iou == argmax_m of
    r = inter * recip((3/13)(a1+a2)) (recip fused into the ScalarE evac).
  - argmax over targets: r cast to bf16, PE-transposed to pred-partition
    space (PSUM), DVE grouped tensor_reduce(max) over the 64-target free
    groups, one-hot mask = (r_T >= best) (exact in bf16; ties averaged via
    a count column), mask PE-transposed back, gather = PE matmul
    (coords hi/lo + count) = GW2^T @ mask.
  - matched flag is NOT carried from the pairwise phase: finalization
    recomputes iou(pred, gathered box) > 0.3, which equals best_iou > 0.3.
  - focal BCE + CIoU finalization in n-partitioned layout, batched over all
    8 images; per-image scalar accumulators reduced via a ones-matmul.
Host combines the 8x8 per-image (focal_sum, masked_ciou_sum, n_pos) triples.
"""

import numpy as np

import concourse.bass as bass
import concourse.bass_isa as bass_isa
from concourse.bacc import Bacc
import concourse.mybir as mybir
from concourse.tile import TileContext

ALU = mybir.AluOpType
ACT = mybir.ActivationFunctionType
F32 = mybir.dt.float32
BF16 = mybir.dt.bfloat16

# problem constants (hardcoded per harness contract)
B_FULL = 64
N = 8400
M = 64
NCORES = 8
BC = B_FULL // NCORES          # images per core
P = 128
C = 66                          # free cols per partition in n-part layout
NPAD = P * C                    # 8448
NC = 1024                       # n-chunk (two PSUM banks of fp32)
CHUNKS = [(k * NC, min(NC, NPAD - k * NC)) for k in range((NPAD + NC - 1) // NC)]
SC13 = 3.0 / 13.0               # iou>0.3  <=>  inter > (3/13)(a1+a2)
EPS = 1e-7


PAD_ROW = np.array([-100.0, -100.0, 1.0, 1.0, -30.0], np.float32)


def pad_preds(preds):
    """Host-side: pad [b, N, 5] -> [b, NPAD, 5] with far-box/low-logit rows."""
    out = np.empty((preds.shape[0], NPAD, 5), np.float32)
    out[:, :N] = preds
    out[:, N:] = PAD_ROW
    return out


def _pred_load(nc, tc, preds_d, PRED, b, bslot, bc):
    """DMA padded preds[b] -> PRED image-slot (n = p*66 + c mapping)."""
    pv = PRED.rearrange("p (b c f) -> p b c f", b=bc, f=5)[:, bslot]  # [128,66,5]
    src = preds_d[b].rearrange("(p c) f -> p c f", c=C)
    nc.gpsimd.dma_start(out=pv[:, :], in_=src)


def host_consts():
    """Host-built constants: selector matmul weights + per-partition scalars."""
    import ml_dtypes
    # K=20 bf16 selector: rows 0..9 hi streams, 10..19 lo streams; stream s
    # picks rows {2s (img A), 2s+1 (img B)} from both halves.
    sels = np.zeros((20, 5 * P), np.float32)
    for s in range(5):
        for base in (0, 10):
            sels[base + 2 * s, s * P : s * P + 64] = 1.0
            sels[base + 2 * s + 1, s * P + 64 : (s + 1) * P] = 1.0
    sels = sels.astype(ml_dtypes.bfloat16)
    onesneg = np.zeros((P, 2), np.float32)
    onesneg[:, 0] = 1.0
    onesneg[:, 1] = -1.0
    identb = np.eye(P, dtype=np.float32).astype(ml_dtypes.bfloat16)
    return sels, onesneg, identb


def build_nc(bc=BC, trn_type=None):
    """Build the per-core Bass program. bc = images per core (even)."""
    pairs = bc // 2
    nc = Bacc() if trn_type is None else Bacc(trn_type=trn_type)
    preds_d = nc.declare_dram_parameter("preds", [bc, NPAD, 5], F32, isOutput=False)
    tgts_d = nc.declare_dram_parameter("targets", [bc, M, 4], F32, isOutput=False)
    sels_d = nc.declare_dram_parameter("sels", [20, 5 * P], BF16, isOutput=False)
    ones_d = nc.declare_dram_parameter("onesneg", [P, 2], F32, isOutput=False)
    identb_d = nc.declare_dram_parameter("identb", [P, P], BF16, isOutput=False)
    out_d = nc.declare_dram_parameter("out", [1, 3 * bc], F32, isOutput=True)

    with TileContext(nc) as tc:
        with (
            tc.tile_pool(name="const", bufs=1) as cpool,
            tc.tile_pool(name="persist", bufs=1) as ppool,
        ):
            # ---- constants (host-supplied) ----
            SELS = cpool.tile([20, 5 * P], BF16, name="SELS")
            nc.sync.dma_start(out=SELS[:, :], in_=sels_d[:, :])
            ON = cpool.tile([P, 2], F32, name="ON")
            nc.sync.dma_start(out=ON[:, :], in_=ones_d[:, :])
            ONES = ON[:, 0:1]
            NEG1 = ON[:, 1:2]
            IDENTB = cpool.tile([P, P], BF16, name="IDENTB")
            nc.sync.dma_start(out=IDENTB[:, :], in_=identb_d[:, :])

            # ---- persistent (all images) ----
            PRED = ppool.tile([P, bc * C * 5], F32, name="PRED")
            X1 = ppool.tile([P, bc * C], F32, name="X1")
            X2 = ppool.tile([P, bc * C], F32, name="X2")
            Y1 = ppool.tile([P, bc * C], F32, name="Y1")
            Y2 = ppool.tile([P, bc * C], F32, name="Y2")
            A1S = ppool.tile([P, bc * C], F32, name="A1S")   # (3/13)*w*h
            # gathered rows per image: 0:4 hi coords, 4 count, 5:9 lo coords,
            # 9 zero  (n = p*66 + c mapping, written back per pair)
            MTF = ppool.tile([P, bc * 10 * C], BF16, name="MTF")
            SC = ppool.tile([P, 3 * bc], F32, name="SC")      # accumulator columns

            with (
                tc.tile_pool(name="stage", bufs=1) as spool,
                tc.tile_pool(name="prep", bufs=2) as qpool,
                tc.tile_pool(name="work", bufs=2) as wpool,
                tc.tile_pool(name="psum", bufs=1, space="PSUM") as pspool,
            ):
                # staging rows, shared across pairs:
                #  STGB rows 0..9: hi(x2A,x2B,x1A,x1B,y2A,y2B,y1A,y1B,a1sA,a1sB)
                #       rows 10..19: bf16 lo residuals of the same
                #  STGF rows 0..19: gathered rows (hi coords+count, lo coords)
                def emit_prep(pr):
                    bA, bB = 2 * pr, 2 * pr + 1
                    STGB = spool.tile([20, NPAD], BF16, name="STGB",
                                      tag="stgb", bufs=2)
                    for bslot in (bA, bB):
                        _pred_load(nc, tc, preds_d, PRED, bslot, bslot, bc)
                    pv = PRED.rearrange("p (b c f) -> p b c f", b=bc, f=5)

                    # per-image coord streams
                    for bslot in (bA, bB):
                        cx = pv[:, bslot, :, 0]
                        cy = pv[:, bslot, :, 1]
                        w = pv[:, bslot, :, 2]
                        h = pv[:, bslot, :, 3]
                        sl = slice(bslot * C, (bslot + 1) * C)
                        WH = qpool.tile([P, C], F32, name="WH", tag="wh", bufs=4)
                        HH = qpool.tile([P, C], F32, name="HH", tag="hh", bufs=4)
                        nc.vector.tensor_scalar(WH[:, :], w, 0.5, None, ALU.mult)
                        nc.vector.tensor_scalar(HH[:, :], h, 0.5, None, ALU.mult)
                        nc.vector.tensor_tensor(X1[:, sl], cx, WH[:, :], ALU.subtract)
                        nc.vector.tensor_tensor(X2[:, sl], cx, WH[:, :], ALU.add)
                        nc.vector.tensor_tensor(Y1[:, sl], cy, HH[:, :], ALU.subtract)
                        nc.vector.tensor_tensor(Y2[:, sl], cy, HH[:, :], ALU.add)
                        nc.vector.scalar_tensor_tensor(
                            A1S[:, sl], w, SC13, h, ALU.mult, ALU.mult
                        )

                    # split to bf16 hi/lo, collapse into rows (n = p*66 + c)
                    for r, T in enumerate((X2, X1, Y2, Y1, A1S)):
                        for j, bslot in enumerate((bA, bB)):
                            tv = T[:, bslot * C : (bslot + 1) * C]
                            THI = qpool.tile([P, C], BF16, name="THI", tag="thi", bufs=4)
                            TLO = qpool.tile([P, C], BF16, name="TLO", tag="tlo", bufs=4)
                            nc.vector.tensor_copy(THI[:, :], tv)
                            nc.vector.tensor_tensor(TLO[:, :], tv, THI[:, :],
                                                    ALU.subtract)
                            for rr, TT_ in ((2 * r + j, THI), (10 + 2 * r + j, TLO)):
                                dst = STGB[rr : rr + 1, :].rearrange(
                                    "o (p c) -> o p c", c=C
                                )
                                nc.sync.dma_start(out=dst, in_=TT_[:, :])

                    # ---- targets: per-partition scalars (A on 0:64, B on 64:128)
                    TGT = qpool.tile([P, 4], F32, name="TGT", tag="tgt", bufs=3)
                    nc.gpsimd.dma_start(out=TGT[0:64, :], in_=tgts_d[bA])
                    nc.gpsimd.dma_start(out=TGT[64:P, :], in_=tgts_d[bB])
                    TWH = qpool.tile([P, 1], F32, name="TWH", tag="twh")
                    THH = qpool.tile([P, 1], F32, name="THH", tag="thh")
                    TX1 = qpool.tile([P, 1], F32, name="TX1", tag="tx1")
                    TY1 = qpool.tile([P, 1], F32, name="TY1", tag="ty1")
                    TX2 = qpool.tile([P, 1], F32, name="TX2", tag="tx2")
                    TY2 = qpool.tile([P, 1], F32, name="TY2", tag="ty2")
                    NX1 = qpool.tile([P, 1], F32, name="NX1", tag="nx1")
                    NY1 = qpool.tile([P, 1], F32, name="NY1", tag="ny1")
                    A2S = qpool.tile([P, 1], F32, name="A2S", tag="a2s")
                    wg = TGT[:, 2:3]
                    hg = TGT[:, 3:4]
                    nc.vector.tensor_scalar(TWH[:, :], wg, 0.5, None, ALU.mult)
                    nc.vector.tensor_scalar(THH[:, :], hg, 0.5, None, ALU.mult)
                    nc.vector.tensor_tensor(TX1[:, :], TGT[:, 0:1], TWH[:, :], ALU.subtract)
                    nc.vector.tensor_tensor(TX2[:, :], TGT[:, 0:1], TWH[:, :], ALU.add)
                    nc.vector.tensor_tensor(TY1[:, :], TGT[:, 1:2], THH[:, :], ALU.subtract)
                    nc.vector.tensor_tensor(TY2[:, :], TGT[:, 1:2], THH[:, :], ALU.add)
                    nc.vector.tensor_scalar(NX1[:, :], TX1[:, :], -1.0, None, ALU.mult)
                    nc.vector.tensor_scalar(NY1[:, :], TY1[:, :], -1.0, None, ALU.mult)
                    nc.vector.scalar_tensor_tensor(
                        A2S[:, :], wg, SC13, hg, ALU.mult, ALU.mult
                    )
                    # gather weights [P, 20] bf16: per image cols {x1,y1,x2,y2,1}
                    # hi at 5j+q (count at 5j+4), lo residuals at 10+5j+q.
                    GW = qpool.tile([P, 10], F32, name="GW", tag="gw", bufs=3)
                    GWB = qpool.tile([P, 20], BF16, name="GWB", tag="gwb", bufs=3)
                    nc.vector.memset(GW[:, :], 0.0)
                    for q, T in enumerate((TX1, TY1, TX2, TY2)):
                        nc.vector.tensor_copy(GW[0:64, q : q + 1], T[0:64, :])
                        nc.vector.tensor_copy(GW[64:P, 5 + q : 6 + q], T[64:P, :])
                    nc.vector.memset(GW[0:64, 4:5], 1.0)
                    nc.vector.memset(GW[64:P, 9:10], 1.0)
                    nc.vector.tensor_copy(GWB[:, 0:10], GW[:, :])
                    nc.vector.tensor_tensor(GWB[:, 10:20], GW[:, :], GWB[:, 0:10],
                                            ALU.subtract)
                    return dict(STGB=STGB, wg=wg, hg=hg, NX1=NX1, NY1=NY1,
                                A2S=A2S, GWB=GWB)

                ctx = emit_prep(0)
                for pr in range(pairs):
                    bA, bB = 2 * pr, 2 * pr + 1
                    STGB = ctx["STGB"]
                    wg, hg = ctx["wg"], ctx["hg"]
                    NX1, NY1 = ctx["NX1"], ctx["NY1"]
                    A2S, GWB = ctx["A2S"], ctx["GWB"]
                    STGF = spool.tile([20, NPAD], BF16, name="STGF",
                                      tag="stgf", bufs=2)
                    # ================= pairwise chunk loop =================
                    for ci, (n0, nc_) in enumerate(CHUNKS):
                        if ci == 5 and pr + 1 < pairs:
                            # overlap next pair's prep with this pair's tail
                            nctx = emit_prep(pr + 1)
                        nbl = nc_ // P  # 128-col transpose blocks in this chunk
                        # PE ones-broadcast of stream rows into PSUM singles
                        PX2 = pspool.tile([P, NC], F32, name="PX2", tag="st", bufs=2)
                        PX1 = pspool.tile([P, NC], F32, name="PX1", tag="st", bufs=2)
                        PY2 = pspool.tile([P, NC], F32, name="PY2", tag="st", bufs=2)
                        PY1 = pspool.tile([P, NC], F32, name="PY1", tag="st", bufs=2)
                        PA1 = pspool.tile([P, NC], F32, name="PA1", tag="st", bufs=2)
                        for j0 in range(0, nc_, 512):
                            jn = min(512, nc_ - j0)
                            rhs = STGB[0:20, n0 + j0 : n0 + j0 + jn]
                            for s, PT_ in enumerate((PX2, PX1, PY2, PY1, PA1)):
                                nc.tensor.matmul(
                                    PT_[:, j0 : j0 + jn],
                                    SELS[:, s * P : (s + 1) * P],
                                    rhs, start=True, stop=True,
                                )
                        # ScalarE: relu with per-partition bias, PSUM -> SBUF
                        AXB = wpool.tile([P, 2 * NC], F32, name="AXB", tag="axb",
                                         bufs=2)
                        AYB = wpool.tile([P, 2 * NC], F32, name="AYB", tag="ayb",
                                         bufs=2)
                        S3 = wpool.tile([P, NC], F32, name="S3", tag="s3", bufs=2)
                        Q = wpool.tile([P, NC], F32, name="Q", tag="q", bufs=2)
                        nc.scalar.activation(AXB[:, 0:nc_], PX2[:, 0:nc_],
                                             ACT.Relu, bias=NX1[:, :])
                        nc.scalar.activation(AXB[:, NC : NC + nc_], PX1[:, 0:nc_],
                                             ACT.Relu, bias=NX1[:, :])
                        nc.scalar.activation(AYB[:, 0:nc_], PY2[:, 0:nc_],
                                             ACT.Relu, bias=NY1[:, :])
                        nc.scalar.activation(AYB[:, NC : NC + nc_], PY1[:, 0:nc_],
                                             ACT.Relu, bias=NY1[:, :])
                        nc.scalar.activation(S3[:, 0:nc_], PA1[:, 0:nc_],
                                             ACT.Identity, bias=A2S[:, :])
                        nc.vector.reciprocal_approx_fast(Q[:, 0:nc_], S3[:, 0:nc_])
                        # DVE: overlaps; GpSimd: relu + inter product
                        CX = wpool.tile([P, NC], F32, name="CX", tag="cx", bufs=2)
                        CY = wpool.tile([P, NC], F32, name="CY", tag="cy", bufs=2)
                        CYR = wpool.tile([P, NC], F32, name="CYR", tag="cyr", bufs=1)
                        INTER = wpool.tile([P, NC], F32, name="INTER", tag="it", bufs=2)
                        RHB = wpool.tile([P, NC], BF16, name="RHB", tag="rh")
                        nc.vector.scalar_tensor_tensor(
                            CX[:, 0:nc_], AXB[:, 0:nc_], wg, AXB[:, NC : NC + nc_],
                            ALU.min, ALU.subtract,
                        )
                        nc.vector.scalar_tensor_tensor(
                            CY[:, 0:nc_], AYB[:, 0:nc_], hg, AYB[:, NC : NC + nc_],
                            ALU.min, ALU.subtract,
                        )
                        nc.scalar.activation(CYR[:, 0:nc_], CY[:, 0:nc_], ACT.Relu)
                        nc.vector.scalar_tensor_tensor(
                            INTER[:, 0:nc_], CX[:, 0:nc_], 0.0, CYR[:, 0:nc_],
                            ALU.max, ALU.mult,
                        )
                        nc.vector.tensor_tensor(RHB[:, 0:nc_], INTER[:, 0:nc_],
                                                Q[:, 0:nc_], ALU.mult)
                        # PE-transpose r (bf16) into pred-partition space
                        RT = pspool.tile([P, NC], BF16, name="RT", tag="rt", bufs=1)
                        for t in range(nbl):
                            nc.tensor.transpose(
                                RT[:, t * P : (t + 1) * P],
                                RHB[:, t * P : (t + 1) * P],
                                IDENTB[:, :],
                            )
                        # best per (pred, image): grouped max over 64 targets
                        BESTC = wpool.tile([P, 2 * NC // P], F32, name="BESTC",
                                           tag="bst")
                        rt4 = RT[:, 0:nc_].rearrange("p (t i m) -> p t i m",
                                                     i=2, m=64)
                        bc3 = BESTC[:, 0 : 2 * nbl].rearrange("p (t i) -> p t i",
                                                              i=2)
                        nc.vector.tensor_reduce(bc3, rt4, mybir.AxisListType.X,
                                                ALU.max)
                        # one-hot (ties allowed; averaged later via count)
                        MASKT = wpool.tile([P, NC], BF16, name="MASKT", tag="mt")
                        mt4 = MASKT[:, 0:nc_].rearrange("p (t i m) -> p t i m",
                                                        i=2, m=64)
                        bb4 = bc3.unsqueeze(3).broadcast_to([P, nbl, 2, 64])
                        nc.vector.tensor_tensor(mt4, rt4, bb4, ALU.is_ge)
                        # transpose mask back to target-partition space
                        MASKM = pspool.tile([P, NC], BF16, name="MASKM", tag="mm",
                                            bufs=1)
                        for t in range(nbl):
                            nc.tensor.transpose(
                                MASKM[:, t * P : (t + 1) * P],
                                MASKT[:, t * P : (t + 1) * P],
                                IDENTB[:, :],
                            )
                        MASKS = wpool.tile([P, NC], BF16, name="MASKS", tag="ms")
                        nc.scalar.activation(MASKS[:, 0:nc_], MASKM[:, 0:nc_],
                                             ACT.Copy)
                        # PE gather: rows = coords hi + count, coords lo
                        GC = pspool.tile([20, NC], F32, name="GC", tag="gc", bufs=1)
                        for j0 in range(0, nc_, 512):
                            jn = min(512, nc_ - j0)
                            nc.tensor.matmul(GC[:, j0 : j0 + jn], GWB[:, :],
                                             MASKS[:, j0 : j0 + jn],
                                             start=True, stop=True)
                        GCB = wpool.tile([20, NC], BF16, name="GCB", tag="gcb",
                                         bufs=2)
                        nc.scalar.activation(GCB[:, 0:nc_], GC[:, 0:nc_], ACT.Copy)
                        nc.sync.dma_start(out=STGF[0:20, n0 : n0 + nc_],
                                          in_=GCB[:, 0:nc_])

                    # ============== return to n-part layout ==============
                    # MTF[p, b, rr, c]: rr 0:4 hi coords, 4 count, 5:9 lo coords
                    mtfv = MTF.rearrange("p (b r c) -> p b r c", b=bc, r=10)
                    for j, bslot in enumerate((bA, bB)):
                        for rr in range(5):
                            src = STGF[5 * j + rr : 5 * j + rr + 1, :].rearrange(
                                "o (p c) -> o p c", c=C)
                            nc.gpsimd.dma_start(out=mtfv[:, bslot, rr], in_=src)
                        for q in range(4):
                            r0 = 10 + 5 * j + q
                            src = STGF[r0 : r0 + 1, :].rearrange(
                                "o (p c) -> o p c", c=C)
                            nc.gpsimd.dma_start(out=mtfv[:, bslot, 5 + q], in_=src)
                    if pr + 1 < pairs:
                        ctx = nctx

            with (
                tc.tile_pool(name="fin", bufs=1) as wpool,
                tc.tile_pool(name="fpsum", bufs=1, space="PSUM") as pspool,
            ):
                # ================= batched finalization =================
                pv = PRED.rearrange("p (b c f) -> p b c f", b=bc, f=5)
                L = pv[:, :, :, 4]      # logits [128, bc, 66]
                CXp = pv[:, :, :, 0]
                CYp = pv[:, :, :, 1]
                Wp = pv[:, :, :, 2]
                Hp = pv[:, :, :, 3]
                BCC = bc * C

                def ftile(name, tag=None, bufs=None):
                    return wpool.tile([P, BCC], F32, name=name, tag=tag or name,
                                      bufs=bufs or 1)

                # ---- matched-target box: (hi + lo) / count ----
                mtfv = MTF.rearrange("p (b r c) -> p b r c", b=bc, r=10)
                CNTV = mtfv[:, :, 4]                      # [P, bc, C]
                CNTF = ftile("CNTF")
                nc.scalar.activation(CNTF.rearrange("p (b c) -> p b c", b=bc),
                                     CNTV, ACT.Copy)
                QCN = ftile("QCN")
                nc.vector.reciprocal_approx_fast(QCN[:, :], CNTF[:, :])
                MTG = wpool.tile([P, bc * 4 * C], F32, name="MTG")
                mtg4 = MTG.rearrange("p (b q c) -> p b q c", b=bc, q=4)
                nc.vector.tensor_tensor(mtg4, mtfv[:, :, 0:4], mtfv[:, :, 5:9],
                                        ALU.add)
                qcb = QCN.rearrange("p (b c) -> p b c", b=bc).unsqueeze(2)
                nc.vector.tensor_tensor(mtg4, mtg4,
                                        qcb.broadcast_to([P, bc, 4, C]), ALU.mult)
                GX1 = mtg4[:, :, 0]
                GY1 = mtg4[:, :, 1]
                GX2 = mtg4[:, :, 2]
                GY2 = mtg4[:, :, 3]
                bview = lambda t: t.rearrange("p (b c) -> p b c", b=bc)

                # ---- intersection with matched boxes + matched flag ----
                T1 = ftile("T1"); T2 = ftile("T2"); T3 = ftile("T3"); T4 = ftile("T4")
                IW = ftile("IW"); IH = ftile("IH"); IN2 = ftile("IN2"); AG = ftile("AG")
                UN = ftile("UN"); QU = ftile("QU"); IOU = ftile("IOU")
                WGE = ftile("WGE"); HGE = ftile("HGE"); A1R = ftile("A1R")
                MTC = ppool.tile([P, BCC], F32, name="MTC")   # matched 0/1

                nc.vector.tensor_tensor(T1[:, :], X1[:, :], GX1, ALU.max)
                nc.vector.tensor_tensor(T2[:, :], X2[:, :], GX2, ALU.min)
                nc.vector.tensor_tensor(IW[:, :], T2[:, :], T1[:, :], ALU.subtract)
                nc.vector.tensor_tensor(T3[:, :], Y1[:, :], GY1, ALU.max)
                nc.vector.tensor_tensor(T4[:, :], Y2[:, :], GY2, ALU.min)
                nc.vector.tensor_tensor(IH[:, :], T4[:, :], T3[:, :], ALU.subtract)
                nc.vector.tensor_scalar(IH[:, :], IH[:, :], 0.0, None, ALU.max)
                nc.vector.scalar_tensor_tensor(IN2[:, :], IW[:, :], 0.0, IH[:, :],
                                               ALU.max, ALU.mult)
                nc.vector.tensor_tensor(WGE[:, :], GX2, GX1, ALU.subtract)
                nc.vector.tensor_tensor(HGE[:, :], GY2, GY1, ALU.subtract)
                nc.vector.tensor_tensor(AG[:, :], WGE[:, :], HGE[:, :], ALU.mult)
                nc.vector.tensor_scalar(A1R[:, :], A1S[:, :], 13.0 / 3.0, None, ALU.mult)
                # matched  <=>  inter > (3/13)(a1 + ag)
                nc.vector.tensor_tensor(T1[:, :], A1R[:, :], AG[:, :], ALU.add)
                nc.vector.scalar_tensor_tensor(MTC[:, :], T1[:, :], SC13, IN2[:, :],
                                               ALU.mult, ALU.is_lt)

                # ---- focal ----
                AZ = ftile("AZ"); SP = ftile("SP"); U0 = ftile("U0"); ZT = ftile("ZT")
                BCE = ftile("BCE"); PT = ftile("PT"); SQ = ftile("SQ"); FF = ftile("FF")
                nc.scalar.activation(AZ[:, :], L, ACT.Abs)
                # softplus(-|z|) = ln(1 + exp(-|z|))
                nc.scalar.activation(SP[:, :], AZ[:, :], ACT.Exp, scale=-1.0)
                nc.scalar.activation(SP[:, :], SP[:, :], ACT.Ln, bias=1.0)
                nc.vector.scalar_tensor_tensor(U0[:, :], L, 0.0, SP[:, :], ALU.max, ALU.add)
                nc.vector.tensor_tensor(ZT[:, :], L, MTC[:, :], ALU.mult)
                nc.vector.tensor_tensor(BCE[:, :], U0[:, :], ZT[:, :], ALU.subtract)
                nc.scalar.activation(PT[:, :], BCE[:, :], ACT.Exp, scale=-1.0)
                nc.scalar.activation(SQ[:, :], PT[:, :], ACT.Square, bias=NEG1[:, :])
                nc.vector.scalar_tensor_tensor(FF[:, :], SQ[:, :], 0.25, BCE[:, :],
                                               ALU.mult, ALU.mult)
                nc.vector.tensor_reduce(SC[:, 0:bc], bview(FF), mybir.AxisListType.X,
                                        ALU.add)

                # ---- CIoU ----
                DX = ftile("DX"); DY = ftile("DY"); DG = ftile("DG"); QD = ftile("QD")
                DD = ftile("DD"); DIOU = ftile("DIOU")
                QH = ftile("QH"); RG = ftile("RG")
                ATG = ftile("ATG"); ATP = ftile("ATP"); VV = ftile("VV"); DEN = ftile("DEN")
                QA = ftile("QA"); AL = ftile("AL"); AV = ftile("AV"); CIO = ftile("CIO")
                MC = ftile("MC")

                # union = a1 + ag - inter
                nc.vector.tensor_tensor(UN[:, :], A1R[:, :], AG[:, :], ALU.add)
                nc.vector.scalar_tensor_tensor(UN[:, :], UN[:, :], EPS, IN2[:, :],
                                               ALU.add, ALU.subtract)
                nc.vector.reciprocal_approx_fast(QU[:, :], UN[:, :])
                nc.vector.tensor_tensor(IOU[:, :], IN2[:, :], QU[:, :], ALU.mult)
                # enclosing diag
                nc.vector.tensor_tensor(T1[:, :], X1[:, :], GX1, ALU.min)
                nc.vector.tensor_tensor(T2[:, :], X2[:, :], GX2, ALU.max)
                nc.vector.tensor_tensor(DX[:, :], T2[:, :], T1[:, :], ALU.subtract)
                nc.vector.tensor_tensor(T3[:, :], Y1[:, :], GY1, ALU.min)
                nc.vector.tensor_tensor(T4[:, :], Y2[:, :], GY2, ALU.max)
                nc.vector.tensor_tensor(DY[:, :], T4[:, :], T3[:, :], ALU.subtract)
                nc.scalar.activation(T1[:, :], DX[:, :], ACT.Square)
                nc.scalar.activation(T2[:, :], DY[:, :], ACT.Square)
                nc.vector.scalar_tensor_tensor(DG[:, :], T1[:, :], EPS, T2[:, :],
                                               ALU.add, ALU.add)
                nc.vector.reciprocal_approx_fast(QD[:, :], DG[:, :])
                # center distance
                nc.vector.tensor_tensor(T3[:, :], GX1, GX2, ALU.add)
                nc.vector.scalar_tensor_tensor(T3[:, :], T3[:, :], 0.5, CXp,
                                               ALU.mult, ALU.subtract)
                nc.vector.tensor_tensor(T4[:, :], GY1, GY2, ALU.add)
                nc.vector.scalar_tensor_tensor(T4[:, :], T4[:, :], 0.5, CYp,
                                               ALU.mult, ALU.subtract)
                nc.scalar.activation(T3[:, :], T3[:, :], ACT.Square)
                nc.scalar.activation(T4[:, :], T4[:, :], ACT.Square)
                nc.vector.tensor_tensor(DD[:, :], T3[:, :], T4[:, :], ALU.add)
                nc.vector.tensor_tensor(DD[:, :], DD[:, :], QD[:, :], ALU.mult)
                # diou - 1 = dist/diag - iou
                nc.vector.scalar_tensor_tensor(DIOU[:, :], IOU[:, :], -1.0, DD[:, :],
                                               ALU.mult, ALU.add)
                # aspect term.  ScalarE Arctan domain is [-pi/2, pi/2], so use
                # arctan(x) = a + 1[x>1]*(pi/2 - 2a),  a = arctan(min(x, 1/x)).
                def atan_pos(dst, x, ta, tb):
                    nc.vector.tensor_scalar(ta[:, :], x[:, :], 1e-20, None, ALU.max)
                    nc.vector.reciprocal_approx_fast(tb[:, :], ta[:, :])
                    nc.vector.tensor_tensor(tb[:, :], ta[:, :], tb[:, :], ALU.min)
                    nc.scalar.activation(dst[:, :], tb[:, :], ACT.Arctan)
                    nc.vector.tensor_scalar(ta[:, :], ta[:, :], 1.0, None, ALU.is_gt)
                    nc.vector.tensor_scalar(tb[:, :], dst[:, :], -2.0, float(np.pi / 2),
                                            ALU.mult, ALU.add)
                    nc.vector.tensor_tensor(ta[:, :], ta[:, :], tb[:, :], ALU.mult)
                    nc.vector.tensor_tensor(dst[:, :], dst[:, :], ta[:, :], ALU.add)

                nc.vector.tensor_scalar(T1[:, :], HGE[:, :], 1e-12, None, ALU.max)
                nc.vector.reciprocal_approx_fast(QH[:, :], T1[:, :])
                nc.vector.tensor_tensor(RG[:, :], WGE[:, :], QH[:, :], ALU.mult)
                atan_pos(ATG, RG, T1, T2)
                nc.vector.tensor_scalar(T2[:, :], Hp, 1e-12, None, ALU.max)
                nc.vector.reciprocal_approx_fast(QH[:, :], T2[:, :])
                nc.vector.scalar_tensor_tensor(RG[:, :], QH[:, :], 1.0, Wp,
                                               ALU.mult, ALU.mult)
                atan_pos(ATP, RG, T1, T2)
                nc.vector.tensor_tensor(T3[:, :], ATG[:, :], ATP[:, :], ALU.subtract)
                nc.scalar.activation(VV[:, :], T3[:, :], ACT.Square,
                                     scale=2.0 / np.pi)
                # alpha = v / (1 - iou + v + eps)
                nc.vector.tensor_tensor(DEN[:, :], VV[:, :], IOU[:, :], ALU.subtract)
                nc.vector.tensor_scalar(DEN[:, :], DEN[:, :], 1.0 + EPS, None, ALU.add)
                nc.vector.reciprocal_approx_fast(QA[:, :], DEN[:, :])
                nc.vector.tensor_tensor(AL[:, :], VV[:, :], QA[:, :], ALU.mult)
                nc.vector.tensor_tensor(AV[:, :], AL[:, :], VV[:, :], ALU.mult)
                # ciou = 1 + (diou - 1) + alpha*v
                nc.vector.scalar_tensor_tensor(CIO[:, :], DIOU[:, :], 1.0, AV[:, :],
                                               ALU.add, ALU.add)
                nc.vector.tensor_tensor(MC[:, :], CIO[:, :], MTC[:, :], ALU.mult)
                nc.vector.tensor_reduce(SC[:, bc : 2 * bc], bview(MC),
                                        mybir.AxisListType.X, ALU.add)
                nc.vector.tensor_reduce(SC[:, 2 * bc : 3 * bc], bview(MTC),
                                        mybir.AxisListType.X, ALU.add)

                # ---- cross-partition reduce + output ----
                PS = pspool.tile([1, 3 * bc], F32, name="PS", tag="ps")
                nc.tensor.matmul(PS[:, :], ONES[:, :], SC[:, :], start=True, stop=True)
                OUTS = wpool.tile([1, 3 * bc], F32, name="OUTS", tag="outs")
                nc.scalar.activation(OUTS[:, :], PS[:, :], ACT.Copy)
                nc.sync.dma_start(out=out_d[:, :], in_=OUTS[:, :])

    nc.finalize()
    return nc


# ---------------- host side ----------------
_CACHE = {}


def _get_nc():
    if "nc" not in _CACHE:
        _CACHE["nc"] = build_nc()
    return _CACHE["nc"]


def combine(per_img):
    """per_img [B, 3] float64: (focal_sum, masked_ciou_sum, n_pos) -> loss."""
    f = per_img[:, 0] / float(N)
    conf = f.mean()
    npos = per_img[:, 2]
    per_box = per_img[:, 1] / np.maximum(npos, 1.0)
    has = (npos > 0).astype(np.float64)
    nimg = has.sum()
    box = (per_box * has).sum() / max(nimg, 1.0)
    return np.float32(conf + 7.5 * box)


def run(preds, targets, **spmd_kwargs):
    from concourse.bass_utils import run_bass_kernel_spmd

    preds = np.ascontiguousarray(preds, np.float32)
    targets = np.ascontiguousarray(targets, np.float32)
    nc = _get_nc()
    sels, onesneg, identb = host_consts()
    in_maps = [
        {
            "preds": pad_preds(preds[c * BC : (c + 1) * BC]),
            "targets": np.ascontiguousarray(targets[c * BC : (c + 1) * BC]),
            "sels": sels,
            "onesneg": onesneg,
            "identb": identb,
        }
        for c in range(NCORES)
    ]
    res = run_bass_kernel_spmd(nc, in_maps, list(range(NCORES)), **spmd_kwargs)
    rows = []
    for c in range(NCORES):
        o = np.asarray(res.results[c]["out"], np.float64).reshape(3, BC)
        rows.append(o.T)  # [BC, 3]
    per_img = np.concatenate(rows, 0)
    return per_img, res


def kernel(preds, targets):
    per_img, _ = run(preds, targets)
    return combine(per_img)
